# revision 1
# baseline (speedup 1.0000x reference)
"""Trainium2 Bass kernel for nn_Net_6maxFull (batch of tiny LSTM chains).

Strategy (pure data parallel over 8 cores, batch on the free axis):
  - 30 LSTM cells scheduled into 12 "slots" of up to 3 independent cells.
  - Per slot one block-diagonal matmul computes all gates:
      lhsT [K, M] host-packed: K = [h-chain rows | x/state rows],
      M = gate rows grouped 32-aligned: i@0, f@32, o@64, g@96.
  - Biases folded into ScalarE activation bias APs.
  - sigmoid(i,f,o) in one ACT instr, tanh(g) in one, c2-tanh in one.
  - c2 = f*c + i*g via two DVE ops using 32-aligned partition groups.
  - h written straight into the next slot's matmul rhs tile; copies of h
    into head-concat tiles go over SBUF->SBUF DMA.
  - Heads (W1/W1o/W2/W3) as small matmuls at end of each batch tile.
"""
import sys
import numpy as np

sys.path.insert(0, "/opt/trn_rl_repo")

B = 131072
NCORE = 8
BC = B // NCORE
H = 10

# slot schedule: list of cells; cell = ("g", layer) or ("o", branch, step)
SLOTS = (
    [[("g", 0), ("o", 0, 0), ("o", 1, 0)],
     [("g", 1), ("o", 0, 1), ("o", 1, 1)],
     [("g", 2), ("o", 0, 2), ("o", 1, 2)],
     [("g", 3), ("o", 0, 3), ("o", 1, 3)],
     [("g", 4), ("o", 2, 0), ("o", 3, 0)],
     [("g", 5), ("o", 2, 1), ("o", 3, 1)],
     [("g", 6), ("o", 2, 2), ("o", 3, 2)],
     [("g", 7), ("o", 2, 3), ("o", 3, 3)],
     [("g", 8), ("o", 4, 0)],
     [("g", 9), ("o", 4, 1)],
     [("o", 4, 2)],
     [("o", 4, 3)]]
)
NSLOT = len(SLOTS)

# gate group partition offsets inside the gates psum/sbuf tiles
GI, GF, GO, GG = 0, 32, 64, 96


def _is_start(cell):
    return (cell[0] == "g" and cell[1] == 0) or (cell[0] == "o" and cell[2] == 0)


def _pred(cell):
    if cell[0] == "g":
        return ("g", cell[1] - 1)
    return ("o", cell[1], cell[2] - 1)


def _x_rows(cell):
    # row range of x^T feeding a chain-start cell
    if cell[0] == "g":
        return (0, 12)
    p = cell[1]
    s = 12 + 5 * p + 1
    return (s, s + 4)


class Plan:
    """Host-side packing plan: row layouts of st/ct blocks and lhsT maps."""

    def __init__(self):
        self.slot = []
        for t, cells in enumerate(SLOTS):
            info = {"cells": cells, "nc": len(cells)}
            info["hp"] = 0 if t == 0 else 10 * len(SLOTS[t - 1])
            # DMA block rows: x rows for start cells then h-state rows per cell
            rows = []  # list of (kind, cell) kind in {x, h}
            for c in cells:
                if _is_start(c):
                    rows.append(("x", c))
            for c in cells:
                rows.append(("h", c))
            info["strows"] = rows
            info["R"] = sum(4 if (k == "x" and c[0] == "o") else
                            12 if (k == "x") else 10 for k, c in rows)
            info["K"] = info["hp"] + info["R"]
            info["M"] = 128  # g-group padded to full 32 rows
            self.slot.append(info)


PLAN = Plan()


def pack_host(inp, np_dt):
    """Build all DRAM-side arrays (full batch; sharding happens later).

    Returns dict name -> np.ndarray. Batch-carrying arrays have shape
    [rows, B]; weights/bias arrays are replicated across cores.
    """
    f32 = np.float32
    out = {}
    Bt = inp["x"].shape[0]
    xT = np.ascontiguousarray(inp["x"].T.astype(np_dt))            # [37, B]
    genh = {i: np.ascontiguousarray(inp["gen_h"][i].T.astype(np_dt)) for i in range(10)}
    genc = {i: np.ascontiguousarray(inp["gen_c"][i].T.astype(np_dt)) for i in range(10)}
    opph = {(p, s): np.ascontiguousarray(inp["opp_h"][p][s].T.astype(np_dt))
            for p in range(5) for s in range(4)}
    oppc = {(p, s): np.ascontiguousarray(inp["opp_c"][p][s].T.astype(np_dt))
            for p in range(5) for s in range(4)}

    def cell_w(cell):
        # returns Wih [40, din], Whh [40, 10], bias [40]
        if cell[0] == "g":
            i = cell[1]
            if i == 0:
                return (inp["W_g0_ih"], inp["W_g0_hh"],
                        inp["b_g0_ih"] + inp["b_g0_hh"])
            return (inp["W_g_ih"][i - 1], inp["W_g_hh"][i - 1],
                    inp["b_g_ih"][i - 1] + inp["b_g_hh"][i - 1])
        p, s = cell[1], cell[2]
        if s == 0:
            return (inp["W_o0_ih"][p], inp["W_o0_hh"][p],
                    inp["b_o0_ih"][p] + inp["b_o0_hh"][p])
        return (inp["W_o_ih"][p][s - 1], inp["W_o_hh"][p][s - 1],
                inp["b_o_ih"][p][s - 1] + inp["b_o_hh"][p][s - 1])

    for t, info in enumerate(PLAN.slot):
        cells = info["cells"]
        # ---- st block [R, B] ----
        st = np.empty((info["R"], Bt), np_dt)
        row_of = {}
        r = 0
        for kind, c in info["strows"]:
            if kind == "x":
                a, b = _x_rows(c)
                st[r:r + (b - a)] = xT[a:b]
                row_of[("x", c)] = r
                r += b - a
            else:
                src = genh[c[1]] if c[0] == "g" else opph[(c[1], c[2])]
                st[r:r + 10] = src
                row_of[("h", c)] = r
                r += 10
        out[f"st{t}"] = st
        # ---- ct block [10*nc, B] ----
        ct = np.empty((10 * info["nc"], Bt), np_dt)
        for k, c in enumerate(cells):
            src = genc[c[1]] if c[0] == "g" else oppc[(c[1], c[2])]
            ct[10 * k:10 * k + 10] = src
        out[f"ct{t}"] = ct
        # ---- lhsT [K, M] and bias [128] ----
        lw = np.zeros((info["K"], info["M"]), f32)
        bias = np.zeros((128, 1), f32)
        prev_cells = SLOTS[t - 1] if t > 0 else []
        for k, c in enumerate(cells):
            Wih, Whh, bvec = cell_w(c)
            Wih = np.asarray(Wih, f32)
            Whh = np.asarray(Whh, f32)
            bvec = np.asarray(bvec, f32)
            # gate row slices in torch order i,f,g,o
            gslice = {"i": slice(0, 10), "f": slice(10, 20),
                      "g": slice(20, 30), "o": slice(30, 40)}
            goff = {"i": GI + 10 * k, "f": GF + 10 * k,
                    "o": GO + 10 * k, "g": GG + 10 * k}
            # tanh-everywhere: sigmoid(x) = (tanh(x/2)+1)/2, so pre-acts of
            # i,f,o are halved; chained h inputs carry h' = 2h, so those
            # columns get an extra 0.5.
            gsc = {"i": 0.5, "f": 0.5, "o": 0.5, "g": 1.0}
            if _is_start(c):
                r0 = info["hp"] + row_of[("x", c)]
                din = Wih.shape[1]
                for gn in "ifog":
                    lw[r0:r0 + din, goff[gn]:goff[gn] + 10] = gsc[gn] * Wih[gslice[gn]].T
            else:
                pos = prev_cells.index(_pred(c))
                r0 = 10 * pos
                for gn in "ifog":
                    lw[r0:r0 + 10, goff[gn]:goff[gn] + 10] = 0.5 * gsc[gn] * Wih[gslice[gn]].T
            # state rows
            r0 = info["hp"] + row_of[("h", c)]
            for gn in "ifog":
                lw[r0:r0 + 10, goff[gn]:goff[gn] + 10] = gsc[gn] * Whh[gslice[gn]].T
                bias[goff[gn]:goff[gn] + 10, 0] = gsc[gn] * bvec[gslice[gn]]
        out[f"lw{t}"] = lw.astype(np_dt)
        out[f"bias{t}"] = bias

    # ---- heads ----
    W1 = np.asarray(inp["W1"], f32)      # [50, 100]
    W1o = np.asarray(inp["W1o"], f32)    # [20, 40]
    W2 = np.asarray(inp["W2"], f32)      # [10, 70]
    W3 = np.asarray(inp["W3"], f32)      # [1, 10]
    out["whg"] = (0.5 * W1.T).copy().astype(np_dt)       # [100, 50]
    who = np.zeros((80, 40), f32)
    for s in range(4):
        blk = 0.5 * W1o[:, 10 * s:10 * s + 10].T         # [10, 20]
        who[20 * s:20 * s + 10, 0:20] = blk
        who[20 * s + 10:20 * s + 20, 20:40] = blk
    out["who01"] = who.astype(np_dt)
    out["who23"] = who.astype(np_dt)
    out["who4"] = (0.5 * W1o.T).copy().astype(np_dt)     # [40, 20]
    out["w2a"] = W2[:, 0:50].T.copy().astype(np_dt)      # [50, 10]
    w2o = (W2[:, 50:70] / 5.0).T                          # [20, 10]
    out["w2b"] = np.vstack([w2o, w2o]).astype(np_dt)     # [40, 10]
    out["w2c"] = w2o.copy().astype(np_dt)                # [20, 10]
    out["w3"] = W3.T.copy().astype(np_dt)                # [10, 1]
    for w_ in (10, 20, 30):
        ia = np.zeros((GF + w_, w_), f32)
        for r in range(w_):
            ia[r, r] = 0.5
            ia[GF + r, r] = 0.5
        out[f"iadd{w_}"] = ia.astype(np_dt)
    hb = np.zeros((128, 8), f32)
    hb[0:50, 0] = np.asarray(inp["b1"], f32)
    hb[0:40, 1] = np.tile(np.asarray(inp["b1o"], f32), 2)
    hb[0:20, 2] = np.asarray(inp["b1o"], f32)
    hb[0:10, 3] = np.asarray(inp["b2"], f32)
    hb[0:1, 4] = np.asarray(inp["b3"], f32)
    out["hbias"] = hb
    return out


def build_nc(Bc, FD, np_dt):
    """Build the SPMD Bass program for one core over Bc batch columns."""
    import concourse.bass as bass
    import concourse.tile as tile
    from concourse import bacc, mybir

    dt = {np.dtype(np.float32): mybir.dt.float32}.get(np.dtype(np_dt))
    if dt is None:
        import ml_dtypes
        assert np.dtype(np_dt) == np.dtype(ml_dtypes.bfloat16)
        dt = mybir.dt.bfloat16
    f32 = mybir.dt.float32
    AF = mybir.ActivationFunctionType

    PSUM_FD = min(1024, FD)
    N_MM = min(512, PSUM_FD)
    n_tiles = Bc // FD
    assert Bc % FD == 0 and FD % PSUM_FD == 0 and PSUM_FD % N_MM == 0

    nc = bacc.Bacc(None, target_bir_lowering=False, debug=False)
    P = PLAN.slot
    dr = {}
    for t in range(NSLOT):
        dr[f"st{t}"] = nc.declare_dram_parameter(f"st{t}", [P[t]["R"], Bc], dt, isOutput=False)
        dr[f"ct{t}"] = nc.declare_dram_parameter(f"ct{t}", [10 * P[t]["nc"], Bc], dt, isOutput=False)
        dr[f"lw{t}"] = nc.declare_dram_parameter(f"lw{t}", [P[t]["K"], P[t]["M"]], dt, isOutput=False)
        dr[f"bias{t}"] = nc.declare_dram_parameter(f"bias{t}", [128, 1], f32, isOutput=False)
    for name, shp in [("whg", [100, 50]), ("who01", [80, 40]), ("who23", [80, 40]),
                      ("who4", [40, 20]), ("w2a", [50, 10]), ("w2b", [40, 10]),
                      ("w2c", [20, 10]), ("w3", [10, 1]),
                      ("iadd10", [42, 10]), ("iadd20", [52, 20]), ("iadd30", [62, 30])]:
        dr[name] = nc.declare_dram_parameter(name, shp, dt, isOutput=False)
    dr["hbias"] = nc.declare_dram_parameter("hbias", [128, 8], f32, isOutput=False)
    out_d = nc.declare_dram_parameter("out", [1, Bc], f32, isOutput=True)

    from contextlib import ExitStack
    with tile.TileContext(nc) as tc:
        with ExitStack() as ctx:
            consts = ctx.enter_context(tc.tile_pool(name="consts", bufs=1))
            rhsp = ctx.enter_context(tc.tile_pool(name="rhs", bufs=7))
            sp = ctx.enter_context(tc.tile_pool(name="sig", bufs=4))
            zp = ctx.enter_context(tc.tile_pool(name="z", bufs=4))
            up = ctx.enter_context(tc.tile_pool(name="u", bufs=3))
            cp = ctx.enter_context(tc.tile_pool(name="c2", bufs=3))
            hp_ = ctx.enter_context(tc.tile_pool(name="hcat", bufs=2))
            fp = ctx.enter_context(tc.tile_pool(name="fh", bufs=1))
            outp = ctx.enter_context(tc.tile_pool(name="outp", bufs=2))
            pg = ctx.enter_context(tc.tile_pool(name="pgate", bufs=2, space="PSUM"))

            # ---- constants ----
            lw = {}
            bias = {}
            for t in range(NSLOT):
                lw[t] = consts.tile([P[t]["K"], P[t]["M"]], dt, tag=f"lw{t}", name=f"lw{t}")
                nc.sync.dma_start(out=lw[t], in_=dr[f"lw{t}"][:])
                bias[t] = consts.tile([128, 1], f32, tag=f"bias{t}", name=f"biast{t}")
                nc.sync.dma_start(out=bias[t], in_=dr[f"bias{t}"][:])
            hw = {}
            for name in ["whg", "who01", "who23", "who4", "w2a", "w2b", "w2c", "w3",
                         "iadd10", "iadd20", "iadd30"]:
                hw[name] = consts.tile(list(dr[name].shape), dt, tag=name, name=f"hw_{name}")
                nc.sync.dma_start(out=hw[name], in_=dr[name][:])
            hb = consts.tile([128, 8], f32, tag="hbias")
            nc.sync.dma_start(out=hb, in_=dr["hbias"][:])


            for it in range(n_tiles):
                col = slice(it * FD, (it + 1) * FD)
                # head concat tiles
                HG = hp_.tile([100, FD], dt, tag="HG")
                HO = {0: hp_.tile([80, FD], dt, tag="HO01", name=f"HO01_{it}"),
                      1: hp_.tile([80, FD], dt, tag="HO23", name=f"HO23_{it}"),
                      2: hp_.tile([40, FD], dt, tag="HO4", name=f"HO4_{it}")}
                rhs = {}
                for t in range(NSLOT + 1):
                    kt = P[t]["K"] if t < NSLOT else 10
                    rhs[t] = rhsp.tile([kt, FD], dt, tag="rhs", name=f"rhs_{it}_{t}")
                # stage first two state DMAs; the rest issue inside the loop
                for t in (0, 1):
                    nc.sync.dma_start(out=rhs[t][P[t]["hp"]:P[t]["K"], :],
                                      in_=dr[f"st{t}"][:, col])

                for t in range(NSLOT):
                    info = P[t]
                    ncell = info["nc"]
                    w = 10 * ncell
                    if t + 2 < NSLOT:
                        t2 = t + 2
                        nc.sync.dma_start(out=rhs[t2][P[t2]["hp"]:P[t2]["K"], :],
                                          in_=dr[f"st{t2}"][:, col])
                    S = sp.tile([128, FD], dt, tag="S", name=f"S_{it}_{t}")
                    Z = zp.tile([GF + 32, FD], dt, tag="Z", name=f"Z_{it}_{t}")
                    U = up.tile([GF + 32, FD], dt, tag="U", name=f"U_{it}_{t}")
                    T2 = cp.tile([GO + 32, FD], dt, tag="T2", name=f"T2_{it}_{t}")
                    # c states -> Z[32:32+w]
                    nc.sync.dma_start(out=Z[GF:GF + w, :], in_=dr[f"ct{t}"][:, col])
                    pt = pg.tile([128, FD], f32, tag="pg", name=f"pg_{it}_{t}")
                    for m in range(FD // N_MM):
                        mcol = slice(m * N_MM, (m + 1) * N_MM)
                        nc.tensor.matmul(pt[:, mcol], lw[t][:],
                                         rhs[t][0:info["K"], mcol],
                                         start=True, stop=True)
                    # tanh over ALL gate groups (i,f,o pre-halved on host)
                    nc.scalar.activation(S[0:128, :], pt[0:128, :],
                                         AF.Tanh, bias=bias[t][0:128])
                    # move tanh(g) next to c for the fused product
                    nc.vector.tensor_copy(Z[0:32, :], S[GG:GG + 32, :])
                    # U = (T_if + 1) * [g | c]
                    nc.vector.scalar_tensor_tensor(
                        U[0:GF + w], S[0:GF + w], 1.0, Z[0:GF + w],
                        mybir.AluOpType.add, mybir.AluOpType.mult)
                    # c2 = 0.5*(row + row+32) back into pt[0:w] (psum reuse)
                    iw = hw[f"iadd{w}"]
                    for m in range(FD // N_MM):
                        mcol = slice(m * N_MM, (m + 1) * N_MM)
                        nc.tensor.matmul(pt[0:w, mcol], iw[:],
                                         U[0:GF + w, mcol],
                                         start=True, stop=True)
                    # T2 = tanh(c2) at base GO (pairs with T_o)
                    nc.scalar.activation(T2[GO:GO + w, :], pt[0:w, :], AF.Tanh)
                    # h' = 2h = (T_o + 1) * tanh(c2) -> next slot rhs rows 0:w
                    nc.vector.scalar_tensor_tensor(
                        rhs[t + 1][0:w, :], S[GO:GO + w, :], 1.0, T2[GO:GO + w, :],
                        mybir.AluOpType.add, mybir.AluOpType.mult)
                    # copy h pieces into head concat tiles (SBUF->SBUF DMA)
                    hsrc = rhs[t + 1]
                    if SLOTS[t][0][0] == "g":
                        gi = SLOTS[t][0][1]
                        nc.sync.dma_start(out=HG[10 * gi:10 * gi + 10, :], in_=hsrc[0:10, :])
                    for k, c in enumerate(SLOTS[t]):
                        if c[0] == "o":
                            p, s = c[1], c[2]
                            pair = p // 2 if p < 4 else 2
                            drow = (20 * s + 10 * (p % 2)) if p < 4 else 10 * s
                            nc.sync.dma_start(out=HO[pair][drow:drow + 10, :],
                                              in_=hsrc[10 * k:10 * k + 10, :])

                # ---- heads ----
                F1 = fp.tile([50, FD], dt, tag="F1", name=f"F1_{it}")
                Fo = {0: fp.tile([40, FD], dt, tag="Fo01", name=f"Fo01_{it}"),
                      1: fp.tile([40, FD], dt, tag="Fo23", name=f"Fo23_{it}"),
                      2: fp.tile([20, FD], dt, tag="Fo4", name=f"Fo4_{it}")}
                F2 = fp.tile([10, FD], dt, tag="F2", name=f"F2_{it}")
                out_sb = outp.tile([1, FD], f32, tag="out", name=f"out_{it}")

                def head_mm(psname, pairs, nrow, bias_ap, Fdst):
                    p_ = pg.tile([128, FD], f32, tag="pg", name=psname)
                    for m in range(FD // N_MM):
                        mc = slice(m * N_MM, (m + 1) * N_MM)
                        for j, (lh, rh) in enumerate(pairs):
                            nc.tensor.matmul(p_[0:nrow, mc], lh[:], rh[:, mc],
                                             start=(j == 0), stop=(j == len(pairs) - 1))
                    nc.scalar.activation(Fdst[0:nrow, :], p_[0:nrow, :],
                                         AF.Tanh, bias=bias_ap)

                head_mm(f"p1_{it}", [(hw["whg"], HG)], 50, hb[0:50, 0:1], F1)
                head_mm(f"po1_{it}", [(hw["who01"], HO[0])], 40, hb[0:40, 1:2], Fo[0])
                head_mm(f"po2_{it}", [(hw["who23"], HO[1])], 40, hb[0:40, 1:2], Fo[1])
                head_mm(f"po3_{it}", [(hw["who4"], HO[2])], 20, hb[0:20, 2:3], Fo[2])
                head_mm(f"p2_{it}",
                        [(hw["w2a"], F1), (hw["w2b"], Fo[0]),
                         (hw["w2b"], Fo[1]), (hw["w2c"], Fo[2])],
                        10, hb[0:10, 3:4], F2)
                p3 = pg.tile([128, FD], f32, tag="pg", name=f"p3_{it}")
                for m in range(FD // N_MM):
                    mc = slice(m * N_MM, (m + 1) * N_MM)
                    nc.tensor.matmul(p3[0:1, mc], hw["w3"][:], F2[:, mc],
                                     start=True, stop=True)
                nc.scalar.activation(out_sb[0:1, :], p3[0:1, :],
                                     AF.Tanh, bias=hb[0:1, 4:5])
                nc.sync.dma_start(out=out_d[0:1, col], in_=out_sb)

    nc.finalize()
    return nc


def kernel(**inputs):
    import ml_dtypes
    np_dt = ml_dtypes.bfloat16
    FD = 2048
    inputs = {k: np.asarray(v) for k, v in inputs.items()}
    packed = pack_host(inputs, np_dt)
    nc = build_nc(BC, FD, np_dt)

    batch_keys = [k for k in packed if k.startswith(("st", "ct"))]
    in_maps = []
    for c in range(NCORE):
        m = {}
        for k, v in packed.items():
            if k in batch_keys:
                m[k] = np.ascontiguousarray(v[:, c * BC:(c + 1) * BC])
            else:
                m[k] = v
        in_maps.append(m)

    from concourse.bass_utils import run_bass_kernel_spmd
    res = run_bass_kernel_spmd(nc, in_maps, list(range(NCORE)))
    outs = [res.results[c]["out"].reshape(-1) for c in range(NCORE)]
    return np.concatenate(outs).reshape(B, 1).astype(np.float32)


if __name__ == "__main__":
    pass



# revision 21
# speedup vs baseline: 3.3844x; 3.3844x over previous
"""Trainium2 Bass kernel for nn_Net_6maxFull (batch of tiny LSTM chains).

Strategy (pure data parallel over 8 cores, batch on the free axis):
  - 30 LSTM cells scheduled into 12 "slots" of up to 3 independent cells.
  - Per slot one block-diagonal matmul computes all gates:
      lhsT [K, M] host-packed: K = [h-chain rows | x/state rows],
      M = gate rows grouped 32-aligned: i@0, f@32, o@64, g@96.
  - Biases folded into ScalarE activation bias APs.
  - sigmoid(i,f,o) in one ACT instr, tanh(g) in one, c2-tanh in one.
  - c2 = f*c + i*g via two DVE ops using 32-aligned partition groups.
  - h written straight into the next slot's matmul rhs tile; copies of h
    into head-concat tiles go over SBUF->SBUF DMA.
  - Heads (W1/W1o/W2/W3) as small matmuls at end of each batch tile.
"""
import sys
import numpy as np

sys.path.insert(0, "/opt/trn_rl_repo")

B = 131072
NCORE = 8
BC = B // NCORE
H = 10

# slot schedule: list of cells; cell = ("g", layer) or ("o", branch, step)
SLOTS_G = (
    [[("g", 0), ("o", 0, 0), ("o", 1, 0)],
     [("g", 1), ("o", 0, 1), ("o", 1, 1)],
     [("g", 2), ("o", 0, 2), ("o", 1, 2)],
     [("g", 3), ("o", 0, 3), ("o", 1, 3)],
     [("g", 4), ("o", 2, 0), ("o", 3, 0)],
     [("g", 5), ("o", 2, 1), ("o", 3, 1)],
     [("g", 6), ("o", 2, 2), ("o", 3, 2)],
     [("g", 7), ("o", 2, 3), ("o", 3, 3)],
     [("g", 8), ("o", 4, 0)],
     [("g", 9), ("o", 4, 1)],
     [("o", 4, 2)],
     [("o", 4, 3)]]
)
NSLOT_G = len(SLOTS_G)

# gate group partition offsets inside the gates psum/sbuf tiles
GI, GF, GO, GG = 0, 32, 64, 96


def _is_start(cell):
    return (cell[0] == "g" and cell[1] == 0) or (cell[0] == "o" and cell[2] == 0)


def _pred(cell):
    if cell[0] == "g":
        return ("g", cell[1] - 1)
    return ("o", cell[1], cell[2] - 1)


def _x_rows(cell):
    # row range of x^T feeding a chain-start cell
    if cell[0] == "g":
        return (0, 12)
    p = cell[1]
    s = 12 + 5 * p + 1
    return (s, s + 4)


class Plan:
    """Host-side packing plan: row layouts of st/ct blocks and lhsT maps."""

    def __init__(self):
        self.slot = []
        for t, cells in enumerate(SLOTS_G):
            info = {"cells": cells, "nc": len(cells)}
            info["hp"] = 0 if t == 0 else 10 * len(SLOTS_G[t - 1])
            # DMA block rows: x rows for start cells then h-state rows per cell
            rows = []  # list of (kind, cell) kind in {x, h}
            for c in cells:
                if _is_start(c):
                    rows.append(("x", c))
            for c in cells:
                rows.append(("h", c))
            info["strows"] = rows
            info["R"] = sum(4 if (k == "x" and c[0] == "o") else
                            12 if (k == "x") else 10 for k, c in rows)
            info["K"] = info["hp"] + info["R"]
            info["M"] = 128  # g-group padded to full 32 rows
            self.slot.append(info)


PLAN = Plan()


def pack_host(inp, np_dt):
    """Build all DRAM-side arrays (full batch; sharding happens later).

    Returns dict name -> np.ndarray. Batch-carrying arrays have shape
    [rows, B]; weights/bias arrays are replicated across cores.
    """
    f32 = np.float32
    out = {}
    Bt = inp["x"].shape[0]
    xT = np.ascontiguousarray(inp["x"].T.astype(np_dt))            # [37, B]
    genh = {i: np.ascontiguousarray(inp["gen_h"][i].T.astype(np_dt)) for i in range(10)}
    genc = {i: np.ascontiguousarray(inp["gen_c"][i].T.astype(np_dt)) for i in range(10)}
    opph = {(p, s): np.ascontiguousarray(inp["opp_h"][p][s].T.astype(np_dt))
            for p in range(5) for s in range(4)}
    oppc = {(p, s): np.ascontiguousarray(inp["opp_c"][p][s].T.astype(np_dt))
            for p in range(5) for s in range(4)}

    def cell_w(cell):
        # returns Wih [40, din], Whh [40, 10], bias [40]
        if cell[0] == "g":
            i = cell[1]
            if i == 0:
                return (inp["W_g0_ih"], inp["W_g0_hh"],
                        inp["b_g0_ih"] + inp["b_g0_hh"])
            return (inp["W_g_ih"][i - 1], inp["W_g_hh"][i - 1],
                    inp["b_g_ih"][i - 1] + inp["b_g_hh"][i - 1])
        p, s = cell[1], cell[2]
        if s == 0:
            return (inp["W_o0_ih"][p], inp["W_o0_hh"][p],
                    inp["b_o0_ih"][p] + inp["b_o0_hh"][p])
        return (inp["W_o_ih"][p][s - 1], inp["W_o_hh"][p][s - 1],
                inp["b_o_ih"][p][s - 1] + inp["b_o_hh"][p][s - 1])

    for t, info in enumerate(PLAN.slot):
        cells = info["cells"]
        # ---- st block [R, B] ----
        st = np.empty((info["R"], Bt), np_dt)
        row_of = {}
        r = 0
        for kind, c in info["strows"]:
            if kind == "x":
                a, b = _x_rows(c)
                st[r:r + (b - a)] = xT[a:b]
                row_of[("x", c)] = r
                r += b - a
            else:
                src = genh[c[1]] if c[0] == "g" else opph[(c[1], c[2])]
                st[r:r + 10] = src
                row_of[("h", c)] = r
                r += 10
        out[f"st{t}"] = st
        # ---- ct block [10*nc, B] ----
        ct = np.empty((10 * info["nc"], Bt), np_dt)
        for k, c in enumerate(cells):
            src = genc[c[1]] if c[0] == "g" else oppc[(c[1], c[2])]
            ct[10 * k:10 * k + 10] = src
        out[f"ct{t}"] = ct
        # ---- lhsT [K, M] and bias [128] ----
        lw = np.zeros((info["K"], info["M"]), f32)
        bias = np.zeros((128, 1), f32)
        prev_cells = SLOTS_G[t - 1] if t > 0 else []
        for k, c in enumerate(cells):
            Wih, Whh, bvec = cell_w(c)
            Wih = np.asarray(Wih, f32)
            Whh = np.asarray(Whh, f32)
            bvec = np.asarray(bvec, f32)
            # gate row slices in torch order i,f,g,o
            gslice = {"i": slice(0, 10), "f": slice(10, 20),
                      "g": slice(20, 30), "o": slice(30, 40)}
            goff = {"i": GI + 10 * k, "f": GF + 10 * k,
                    "o": GO + 10 * k, "g": GG + 10 * k}
            # tanh-everywhere: sigmoid(x) = (tanh(x/2)+1)/2, so pre-acts of
            # i,f,o are halved; chained h inputs carry h' = 2h, so those
            # columns get an extra 0.5.
            gsc = {"i": 0.5, "f": 0.5, "o": 0.5, "g": 1.0}
            if _is_start(c):
                r0 = info["hp"] + row_of[("x", c)]
                din = Wih.shape[1]
                for gn in "ifog":
                    lw[r0:r0 + din, goff[gn]:goff[gn] + 10] = gsc[gn] * Wih[gslice[gn]].T
            else:
                pos = prev_cells.index(_pred(c))
                r0 = 10 * pos
                for gn in "ifog":
                    lw[r0:r0 + 10, goff[gn]:goff[gn] + 10] = 0.5 * gsc[gn] * Wih[gslice[gn]].T
            # state rows
            r0 = info["hp"] + row_of[("h", c)]
            for gn in "ifog":
                lw[r0:r0 + 10, goff[gn]:goff[gn] + 10] = gsc[gn] * Whh[gslice[gn]].T
                bias[goff[gn]:goff[gn] + 10, 0] = gsc[gn] * bvec[gslice[gn]]
        out[f"lw{t}"] = lw.astype(np_dt)
        out[f"bias{t}"] = bias

    # ---- heads ----
    W1 = np.asarray(inp["W1"], f32)      # [50, 100]
    W1o = np.asarray(inp["W1o"], f32)    # [20, 40]
    W2 = np.asarray(inp["W2"], f32)      # [10, 70]
    W3 = np.asarray(inp["W3"], f32)      # [1, 10]
    out["whg"] = (0.5 * W1.T).copy().astype(np_dt)       # [100, 50]
    who = np.zeros((80, 40), f32)
    for s in range(4):
        blk = 0.5 * W1o[:, 10 * s:10 * s + 10].T         # [10, 20]
        who[20 * s:20 * s + 10, 0:20] = blk
        who[20 * s + 10:20 * s + 20, 20:40] = blk
    out["who01"] = who.astype(np_dt)
    out["who23"] = who.astype(np_dt)
    out["who4"] = (0.5 * W1o.T).copy().astype(np_dt)     # [40, 20]
    out["w2a"] = W2[:, 0:50].T.copy().astype(np_dt)      # [50, 10]
    w2o = (W2[:, 50:70] / 5.0).T                          # [20, 10]
    out["w2b"] = np.vstack([w2o, w2o]).astype(np_dt)     # [40, 10]
    out["w2c"] = w2o.copy().astype(np_dt)                # [20, 10]
    out["w3"] = W3.T.copy().astype(np_dt)                # [10, 1]
    for w_ in (10, 20, 30):
        ia = np.zeros((GF + w_, w_), f32)
        for r in range(w_):
            ia[r, r] = 0.5
            ia[GF + r, r] = 0.5
        out[f"iadd{w_}"] = ia.astype(np_dt)
    hb = np.zeros((128, 8), f32)
    hb[0:50, 0] = np.asarray(inp["b1"], f32)
    hb[0:40, 1] = np.tile(np.asarray(inp["b1o"], f32), 2)
    hb[0:20, 2] = np.asarray(inp["b1o"], f32)
    hb[0:10, 3] = np.asarray(inp["b2"], f32)
    hb[0:1, 4] = np.asarray(inp["b3"], f32)
    out["hbias"] = hb
    return out


def build_nc(Bc, FD, np_dt):
    """Build the SPMD Bass program for one core over Bc batch columns."""
    import concourse.bass as bass
    import concourse.tile as tile
    from concourse import bacc, mybir

    dt = {np.dtype(np.float32): mybir.dt.float32}.get(np.dtype(np_dt))
    if dt is None:
        import ml_dtypes
        assert np.dtype(np_dt) == np.dtype(ml_dtypes.bfloat16)
        dt = mybir.dt.bfloat16
    f32 = mybir.dt.float32
    AF = mybir.ActivationFunctionType

    PSUM_FD = min(1024, FD)
    N_MM = min(512, PSUM_FD)
    n_tiles = Bc // FD
    assert Bc % FD == 0 and FD % PSUM_FD == 0 and PSUM_FD % N_MM == 0

    nc = bacc.Bacc(None, target_bir_lowering=False, debug=False)
    P = PLAN.slot
    dr = {}
    for t in range(NSLOT_G):
        dr[f"st{t}"] = nc.declare_dram_parameter(f"st{t}", [P[t]["R"], Bc], dt, isOutput=False)
        dr[f"ct{t}"] = nc.declare_dram_parameter(f"ct{t}", [10 * P[t]["nc"], Bc], dt, isOutput=False)
        dr[f"lw{t}"] = nc.declare_dram_parameter(f"lw{t}", [P[t]["K"], P[t]["M"]], dt, isOutput=False)
        dr[f"bias{t}"] = nc.declare_dram_parameter(f"bias{t}", [128, 1], f32, isOutput=False)
    for name, shp in [("whg", [100, 50]), ("who01", [80, 40]), ("who23", [80, 40]),
                      ("who4", [40, 20]), ("w2a", [50, 10]), ("w2b", [40, 10]),
                      ("w2c", [20, 10]), ("w3", [10, 1]),
                      ("iadd10", [42, 10]), ("iadd20", [52, 20]), ("iadd30", [62, 30])]:
        dr[name] = nc.declare_dram_parameter(name, shp, dt, isOutput=False)
    dr["hbias"] = nc.declare_dram_parameter("hbias", [128, 8], f32, isOutput=False)
    out_d = nc.declare_dram_parameter("out", [1, Bc], f32, isOutput=True)

    from contextlib import ExitStack
    with tile.TileContext(nc) as tc:
        with ExitStack() as ctx:
            consts = ctx.enter_context(tc.tile_pool(name="consts", bufs=1))
            rhsp = ctx.enter_context(tc.tile_pool(name="rhs", bufs=7))
            sp = ctx.enter_context(tc.tile_pool(name="sig", bufs=4))
            zp = ctx.enter_context(tc.tile_pool(name="z", bufs=4))
            up = ctx.enter_context(tc.tile_pool(name="u", bufs=3))
            cp = ctx.enter_context(tc.tile_pool(name="c2", bufs=3))
            hp_ = ctx.enter_context(tc.tile_pool(name="hcat", bufs=2))
            fp = ctx.enter_context(tc.tile_pool(name="fh", bufs=1))
            outp = ctx.enter_context(tc.tile_pool(name="outp", bufs=2))
            pg = ctx.enter_context(tc.tile_pool(name="pgate", bufs=2, space="PSUM"))

            # ---- constants ----
            lw = {}
            bias = {}
            for t in range(NSLOT_G):
                lw[t] = consts.tile([P[t]["K"], P[t]["M"]], dt, tag=f"lw{t}", name=f"lw{t}")
                nc.sync.dma_start(out=lw[t], in_=dr[f"lw{t}"][:])
                bias[t] = consts.tile([128, 1], f32, tag=f"bias{t}", name=f"biast{t}")
                nc.sync.dma_start(out=bias[t], in_=dr[f"bias{t}"][:])
            hw = {}
            for name in ["whg", "who01", "who23", "who4", "w2a", "w2b", "w2c", "w3",
                         "iadd10", "iadd20", "iadd30"]:
                hw[name] = consts.tile(list(dr[name].shape), dt, tag=name, name=f"hw_{name}")
                nc.sync.dma_start(out=hw[name], in_=dr[name][:])
            hb = consts.tile([128, 8], f32, tag="hbias")
            nc.sync.dma_start(out=hb, in_=dr["hbias"][:])


            for it in range(n_tiles):
                col = slice(it * FD, (it + 1) * FD)
                # head concat tiles
                HG = hp_.tile([100, FD], dt, tag="HG")
                HO = {0: hp_.tile([80, FD], dt, tag="HO01", name=f"HO01_{it}"),
                      1: hp_.tile([80, FD], dt, tag="HO23", name=f"HO23_{it}"),
                      2: hp_.tile([40, FD], dt, tag="HO4", name=f"HO4_{it}")}
                rhs = {}
                for t in range(NSLOT_G + 1):
                    kt = P[t]["K"] if t < NSLOT else 10
                    rhs[t] = rhsp.tile([kt, FD], dt, tag="rhs", name=f"rhs_{it}_{t}")
                # stage first two state DMAs; the rest issue inside the loop
                for t in (0, 1):
                    nc.sync.dma_start(out=rhs[t][P[t]["hp"]:P[t]["K"], :],
                                      in_=dr[f"st{t}"][:, col])

                for t in range(NSLOT_G):
                    info = P[t]
                    ncell = info["nc"]
                    w = 10 * ncell
                    if t + 2 < NSLOT:
                        t2 = t + 2
                        nc.sync.dma_start(out=rhs[t2][P[t2]["hp"]:P[t2]["K"], :],
                                          in_=dr[f"st{t2}"][:, col])
                    S = sp.tile([128, FD], dt, tag="S", name=f"S_{it}_{t}")
                    Z = zp.tile([GF + 32, FD], dt, tag="Z", name=f"Z_{it}_{t}")
                    U = up.tile([GF + 32, FD], dt, tag="U", name=f"U_{it}_{t}")
                    T2 = cp.tile([GO + 32, FD], dt, tag="T2", name=f"T2_{it}_{t}")
                    # c states -> Z[32:32+w]
                    nc.sync.dma_start(out=Z[GF:GF + w, :], in_=dr[f"ct{t}"][:, col])
                    pt = pg.tile([128, FD], f32, tag="pg", name=f"pg_{it}_{t}")
                    for m in range(FD // N_MM):
                        mcol = slice(m * N_MM, (m + 1) * N_MM)
                        nc.tensor.matmul(pt[:, mcol], lw[t][:],
                                         rhs[t][0:info["K"], mcol],
                                         start=True, stop=True)
                    # tanh over ALL gate groups (i,f,o pre-halved on host)
                    nc.scalar.activation(S[0:128, :], pt[0:128, :],
                                         AF.Tanh, bias=bias[t][0:128])
                    # move tanh(g) next to c for the fused product
                    nc.vector.tensor_copy(Z[0:32, :], S[GG:GG + 32, :])
                    # U = (T_if + 1) * [g | c]
                    nc.vector.scalar_tensor_tensor(
                        U[0:GF + w], S[0:GF + w], 1.0, Z[0:GF + w],
                        mybir.AluOpType.add, mybir.AluOpType.mult)
                    # c2 = 0.5*(row + row+32) back into pt[0:w] (psum reuse)
                    iw = hw[f"iadd{w}"]
                    for m in range(FD // N_MM):
                        mcol = slice(m * N_MM, (m + 1) * N_MM)
                        nc.tensor.matmul(pt[0:w, mcol], iw[:],
                                         U[0:GF + w, mcol],
                                         start=True, stop=True)
                    # T2 = tanh(c2) at base GO (pairs with T_o)
                    nc.scalar.activation(T2[GO:GO + w, :], pt[0:w, :], AF.Tanh)
                    # h' = 2h = (T_o + 1) * tanh(c2) -> next slot rhs rows 0:w
                    nc.vector.scalar_tensor_tensor(
                        rhs[t + 1][0:w, :], S[GO:GO + w, :], 1.0, T2[GO:GO + w, :],
                        mybir.AluOpType.add, mybir.AluOpType.mult)
                    # copy h pieces into head concat tiles (SBUF->SBUF DMA)
                    hsrc = rhs[t + 1]
                    if SLOTS_G[t][0][0] == "g":
                        gi = SLOTS_G[t][0][1]
                        nc.sync.dma_start(out=HG[10 * gi:10 * gi + 10, :], in_=hsrc[0:10, :])
                    for k, c in enumerate(SLOTS_G[t]):
                        if c[0] == "o":
                            p, s = c[1], c[2]
                            pair = p // 2 if p < 4 else 2
                            drow = (20 * s + 10 * (p % 2)) if p < 4 else 10 * s
                            nc.sync.dma_start(out=HO[pair][drow:drow + 10, :],
                                              in_=hsrc[10 * k:10 * k + 10, :])

                # ---- heads ----
                F1 = fp.tile([50, FD], dt, tag="F1", name=f"F1_{it}")
                Fo = {0: fp.tile([40, FD], dt, tag="Fo01", name=f"Fo01_{it}"),
                      1: fp.tile([40, FD], dt, tag="Fo23", name=f"Fo23_{it}"),
                      2: fp.tile([20, FD], dt, tag="Fo4", name=f"Fo4_{it}")}
                F2 = fp.tile([10, FD], dt, tag="F2", name=f"F2_{it}")
                out_sb = outp.tile([1, FD], f32, tag="out", name=f"out_{it}")

                def head_mm(psname, pairs, nrow, bias_ap, Fdst):
                    p_ = pg.tile([128, FD], f32, tag="pg", name=psname)
                    for m in range(FD // N_MM):
                        mc = slice(m * N_MM, (m + 1) * N_MM)
                        for j, (lh, rh) in enumerate(pairs):
                            nc.tensor.matmul(p_[0:nrow, mc], lh[:], rh[:, mc],
                                             start=(j == 0), stop=(j == len(pairs) - 1))
                    nc.scalar.activation(Fdst[0:nrow, :], p_[0:nrow, :],
                                         AF.Tanh, bias=bias_ap)

                head_mm(f"p1_{it}", [(hw["whg"], HG)], 50, hb[0:50, 0:1], F1)
                head_mm(f"po1_{it}", [(hw["who01"], HO[0])], 40, hb[0:40, 1:2], Fo[0])
                head_mm(f"po2_{it}", [(hw["who23"], HO[1])], 40, hb[0:40, 1:2], Fo[1])
                head_mm(f"po3_{it}", [(hw["who4"], HO[2])], 20, hb[0:20, 2:3], Fo[2])
                head_mm(f"p2_{it}",
                        [(hw["w2a"], F1), (hw["w2b"], Fo[0]),
                         (hw["w2b"], Fo[1]), (hw["w2c"], Fo[2])],
                        10, hb[0:10, 3:4], F2)
                p3 = pg.tile([128, FD], f32, tag="pg", name=f"p3_{it}")
                for m in range(FD // N_MM):
                    mc = slice(m * N_MM, (m + 1) * N_MM)
                    nc.tensor.matmul(p3[0:1, mc], hw["w3"][:], F2[:, mc],
                                     start=True, stop=True)
                nc.scalar.activation(out_sb[0:1, :], p3[0:1, :],
                                     AF.Tanh, bias=hb[0:1, 4:5])
                nc.sync.dma_start(out=out_d[0:1, col], in_=out_sb)

    nc.finalize()
    return nc




# ============ fast path (zero initial states) ============

TW = 4096            # tile width (super-chunk)
CW = 1024            # compute width (sub-chunk)
MW = 512             # matmul moving width
NSC = BC // TW       # 4 super-chunks
PHASES = [[0, 1], [2, 3]]
INTERLEAVE_HEADS = True

# rotation bases per sub-chunk j (o-group must sit at 32*j)
OBASE = [0, 32, 64, 96]
IBASE = [32, 0, 0, 0]
GBASE = [64, 64, 32, 32]

# cells: ("g", layer) or ("o", branch, step)
SLOT_CELLS = [
    [("g", 0), ("o", 0, 0), ("o", 1, 0)],
    [("g", 1), ("o", 0, 1), ("o", 1, 1)],
    [("g", 2), ("o", 0, 2), ("o", 1, 2)],
    [("g", 3), ("o", 0, 3), ("o", 1, 3)],
    [("g", 4), ("o", 2, 0), ("o", 3, 0)],
    [("g", 5), ("o", 2, 1), ("o", 3, 1)],
    [("g", 6), ("o", 2, 2), ("o", 3, 2)],
    [("g", 7), ("o", 2, 3), ("o", 4, 0)],
    [("g", 8), ("o", 3, 3), ("o", 4, 1)],
    [("o", 4, 2), ("g", 9)],
    [("o", 4, 3)],
]
NSLOT = 11
# per-slot: input row range (start,len) in rhs[t] for each cell
SLOT_INROWS = [
    [(0, 12), (12, 4), (16, 4)],
    [(0, 10), (10, 10), (20, 10)],
    [(0, 10), (10, 10), (20, 10)],
    [(0, 10), (10, 10), (20, 10)],
    [(0, 10), (30, 4), (34, 4)],
    [(0, 10), (10, 10), (20, 10)],
    [(0, 10), (10, 10), (20, 10)],
    [(0, 10), (10, 10), (30, 4)],
    [(0, 10), (30, 10), (20, 10)],
    [(20, 10), (0, 10)],
    [(0, 10)],
]
SLOT_K = [20, 30, 30, 30, 38, 30, 30, 34, 40, 30, 10]
RHS_ROWS = [20, 30, 30, 30, 38, 30, 30, 34, 40, 30, 20, 10]
# x DMA per slot: (dst_row_in_rhs, src_row_in_xT32, nrows)
SLOT_XDMA = {0: (0, 0, 20), 4: (30, 20, 8), 7: (30, 28, 4)}
# concat: per slot list of (src_row, n, dst_name, dst_row)
CONCAT = [
    [(0, 10, "HG", 0), (10, 10, "HOa", 0), (20, 10, "HOa", 40)],
    [(0, 10, "HG", 10), (10, 10, "HOa", 10), (20, 10, "HOa", 50)],
    [(0, 10, "HG", 20), (10, 10, "HOa", 20), (20, 10, "HOa", 60)],
    [(0, 10, "HG", 30), (10, 10, "HOa", 30), (20, 10, "HOa", 70)],
    [(0, 10, "HG", 40), (10, 10, "HOb", 0), (20, 10, "HOb", 40)],
    [(0, 10, "HG", 50), (10, 10, "HOb", 10), (20, 10, "HOb", 50)],
    [(0, 10, "HG", 60), (10, 10, "HOb", 20), (20, 10, "HOb", 60)],
    [(0, 10, "HG", 70), (10, 10, "HOb", 30), (20, 10, "HOa", 80)],
    [(0, 10, "HG", 80), (10, 10, "HOb", 70), (20, 10, "HOa", 90)],
    [(0, 10, "HOb", 80), (10, 10, "HG", 90)],
    [(0, 10, "HOb", 90)],
]


def _w(t):
    return 10 * len(SLOT_CELLS[t])


def _span(t, j):
    return max(IBASE[j], OBASE[j], GBASE[j]) + _w(t)


def _cell_w(inp, cell):
    f32 = np.float32
    if cell[0] == "g":
        i = cell[1]
        if i == 0:
            W = np.asarray(inp["W_g0_ih"], f32)
            b = np.asarray(inp["b_g0_ih"], f32) + np.asarray(inp["b_g0_hh"], f32)
        else:
            W = np.asarray(inp["W_g_ih"][i - 1], f32)
            b = np.asarray(inp["b_g_ih"][i - 1], f32) + np.asarray(inp["b_g_hh"][i - 1], f32)
    else:
        p, s = cell[1], cell[2]
        if s == 0:
            W = np.asarray(inp["W_o0_ih"][p], f32)
            b = np.asarray(inp["b_o0_ih"][p], f32) + np.asarray(inp["b_o0_hh"][p], f32)
        else:
            W = np.asarray(inp["W_o_ih"][p][s - 1], f32)
            b = np.asarray(inp["b_o_ih"][p][s - 1], f32) + np.asarray(inp["b_o_hh"][p][s - 1], f32)
    return W, b


def pack_fast(inp, np_dt):
    """Host-side constant packing. Returns dict name -> np.ndarray.
    xT32 is [32, B] (batch-carrying, split per core later)."""
    f32 = np.float32
    out = {}
    x = np.asarray(inp["x"], f32)
    # rows: gen 0:12, then branch p active-stripped cols (4 each)
    xT = np.empty((32, x.shape[0]), f32)
    xT[0:12] = x[:, 0:12].T
    for p in range(5):
        s = 12 + 5 * p + 1
        xT[12 + 4 * p: 16 + 4 * p] = x[:, s:s + 4].T
    out["xT32"] = xT.astype(np_dt)

    for t in range(NSLOT):
        cells = SLOT_CELLS[t]
        w = _w(t)
        K = SLOT_K[t]
        bias = np.zeros((128, 4), f32)
        for j in range(4):
            lw = np.zeros((K, _span(t, j)), f32)
            for k, cell in enumerate(cells):
                W, b = _cell_w(inp, cell)
                r0, rn = SLOT_INROWS[t][k]
                Wi, Wg, Wo = W[0:10], W[20:30], W[30:40]
                bi, bg, bo = b[0:10], b[20:30], b[30:40]
                lw[r0:r0 + rn, IBASE[j] + 10 * k: IBASE[j] + 10 * k + 10] = Wi[:, 0:rn].T
                lw[r0:r0 + rn, OBASE[j] + 10 * k: OBASE[j] + 10 * k + 10] = Wo[:, 0:rn].T
                lw[r0:r0 + rn, GBASE[j] + 10 * k: GBASE[j] + 10 * k + 10] = 2.0 * Wg[:, 0:rn].T
                bias[IBASE[j] + 10 * k: IBASE[j] + 10 * k + 10, j] = bi
                bias[OBASE[j] + 10 * k: OBASE[j] + 10 * k + 10, j] = bo
                bias[GBASE[j] + 10 * k: GBASE[j] + 10 * k + 10, j] = 2.0 * bg
            out[f"lw{t}_{j}"] = lw.astype(np_dt)
        out[f"bias{t}"] = bias

    W1 = np.asarray(inp["W1"], f32)      # [50, 100]
    W1o = np.asarray(inp["W1o"], f32)    # [20, 40]
    W2 = np.asarray(inp["W2"], f32)      # [10, 70]
    W3 = np.asarray(inp["W3"], f32)      # [1, 10]
    out["w1T"] = W1.T.copy().astype(np_dt)               # [100, 50]
    poA = np.zeros((100, 100), f32)
    poB = np.zeros((100, 100), f32)
    poA[0:40, 0:20] = W1o.T
    poA[40:80, 20:40] = W1o.T
    poA[80:100, 80:100] = W1o[:, 0:20].T
    poB[0:40, 40:60] = W1o.T
    poB[40:80, 60:80] = W1o.T
    poB[80:100, 80:100] = W1o[:, 20:40].T
    out["poA"] = poA.astype(np_dt)
    out["poB"] = poB.astype(np_dt)
    out["w2aT"] = W2[:, 0:50].T.copy().astype(np_dt)     # [50, 10]
    w2b = (W2[:, 50:70] / 5.0).T                          # [20, 10]
    out["w2bT"] = np.tile(w2b, (5, 1)).astype(np_dt)     # [100, 10]
    w3rep = np.zeros((74, 1), f32)
    for i in range(3):
        w3rep[32 * i:32 * i + 10, 0] = W3[0]
    out["w3rep"] = w3rep.astype(np_dt)
    hb = np.zeros((128, 4), f32)
    hb[0:50, 0] = np.asarray(inp["b1"], f32)
    hb[0:100, 1] = np.tile(np.asarray(inp["b1o"], f32), 5)
    for i in range(3):
        hb[32 * i:32 * i + 10, 2] = np.asarray(inp["b2"], f32)
        hb[32 * i, 3] = np.asarray(inp["b3"], f32)[0]
    out["hbias"] = hb
    # ---- batch all constants into two arrays (2 DMAs at startup) ----
    coff, ncol = _const_layout()
    CB = np.zeros((128, ncol), np_dt)
    for t in range(NSLOT):
        for j in range(4):
            a = out.pop(f"lw{t}_{j}")
            CB[0:a.shape[0], coff[(t, j)]:coff[(t, j)] + a.shape[1]] = a
    for name in ["w1T", "poA", "poB", "w2aT", "w2bT", "w3rep"]:
        a = out.pop(name)
        CB[0:a.shape[0], coff[name]:coff[name] + a.shape[1]] = a
    CF = np.zeros((128, 48), f32)
    for t in range(NSLOT):
        CF[:, 4 * t:4 * t + 4] = out.pop(f"bias{t}")
    CF[:, 44:48] = out.pop("hbias")
    out["CB"] = CB
    out["CF"] = CF
    return out


def _const_layout():
    off = {}
    col = 0
    for t in range(NSLOT):
        for j in range(4):
            off[(t, j)] = col
            col += _span(t, j)
    for name, w_ in [("w1T", 50), ("poA", 100), ("poB", 100),
                     ("w2aT", 10), ("w2bT", 10), ("w3rep", 1)]:
        off[name] = col
        col += w_
    return off, col


def build_fast(Bc, np_dt):
    import concourse.tile as tile
    from concourse import bacc, mybir

    dt = mybir.dt.float32
    if np.dtype(np_dt) != np.dtype(np.float32):
        dt = mybir.dt.bfloat16
    f32 = mybir.dt.float32
    AF = mybir.ActivationFunctionType
    ALU = mybir.AluOpType

    assert Bc == NSC * TW
    coff, ncol = _const_layout()

    nc = bacc.Bacc(None, target_bir_lowering=False, debug=False)
    dr = {}
    dr["CB"] = nc.declare_dram_parameter("CB", [128, ncol], dt, isOutput=False)
    dr["CF"] = nc.declare_dram_parameter("CF", [128, 48], f32, isOutput=False)
    dr["xT32"] = nc.declare_dram_parameter("xT32", [32, Bc], dt, isOutput=False)
    out_d = nc.declare_dram_parameter("out16", [NSC * 4, CW], f32, isOutput=True)

    from contextlib import ExitStack
    with tile.TileContext(nc) as tc:
        with ExitStack() as ctx:
            consts = ctx.enter_context(tc.tile_pool(name="consts", bufs=1))
            rhsp = ctx.enter_context(tc.tile_pool(name="rhs", bufs=6))
            sp = ctx.enter_context(tc.tile_pool(name="S", bufs=8))
            gp = ctx.enter_context(tc.tile_pool(name="G", bufs=6))
            u4p = ctx.enter_context(tc.tile_pool(name="U4", bufs=3))
            t4p = ctx.enter_context(tc.tile_pool(name="T4", bufs=3))
            hcat = ctx.enter_context(tc.tile_pool(name="hcat", bufs=2))
            f1p = ctx.enter_context(tc.tile_pool(name="F1", bufs=8))
            fop = ctx.enter_context(tc.tile_pool(name="Fo", bufs=8))
            f2p = ctx.enter_context(tc.tile_pool(name="F2", bufs=3))
            outp = ctx.enter_context(tc.tile_pool(name="outp", bufs=2))
            pg = ctx.enter_context(tc.tile_pool(name="pg", bufs=4, space="PSUM"))

            CB = consts.tile([128, ncol], dt, tag="CB")
            nc.sync.dma_start(out=CB, in_=dr["CB"][:])
            CF = consts.tile([128, 48], f32, tag="CF")
            nc.sync.dma_start(out=CF, in_=dr["CF"][:])

            def lw_ap(t, j, K=None):
                o = coff[(t, j)]
                return CB[0:(K or SLOT_K[t]), o:o + _span(t, j)]

            def hw_ap(name, rows, colw):
                o = coff[name]
                return CB[0:rows, o:o + colw]

            def bias_ap(t, j, span):
                return CF[0:span, 4 * t + j:4 * t + j + 1]

            def hb_ap(rows, col):
                return CF[0:rows, 44 + col:44 + col + 1]

            state = {}

            def emit_slot(ph, phase, t, rhs, hgt):
                w = _w(t)
                K = SLOT_K[t]

                def get_rhs(t1, sc):
                    if (t1, sc) not in rhs:
                        rhs[(t1, sc)] = rhsp.tile([RHS_ROWS[t1], TW], dt, tag="rhs",
                                                  name=f"rhs_{ph}_{t1}_{sc}")
                    return rhs[(t1, sc)]

                for sc in phase:
                    get_rhs(t + 1, sc)
                    if (t + 1) in SLOT_XDMA:
                        dst, src, n = SLOT_XDMA[t + 1]
                        nc.sync.dma_start(
                            out=rhs[(t + 1, sc)][dst:dst + n, :],
                            in_=dr["xT32"][src:src + n, sc * TW:(sc + 1) * TW])
                pts = {}
                for j in range(4):
                    span = _span(t, j)
                    for sc in phase:
                        pt = pg.tile([128, CW], f32, tag="pg", name=f"pt_{ph}_{t}_{sc}_{j}")
                        pts[(sc, j)] = pt
                        for m in range(CW // MW):
                            col = slice(j * CW + m * MW, j * CW + (m + 1) * MW)
                            nc.tensor.matmul(pt[0:span, m * MW:(m + 1) * MW],
                                             lw_ap(t, j),
                                             rhs[(t, sc)][0:K, col],
                                             start=True, stop=True)
                Ss = {}
                for sc in phase:
                    for j in range(4):
                        span = _span(t, j)
                        S = sp.tile([128, CW], dt, tag="S", name=f"S_{ph}_{t}_{sc}_{j}")
                        Ss[(sc, j)] = S
                        nc.scalar.activation(S[0:span, :], pts[(sc, j)][0:span, :],
                                             AF.Sigmoid, bias=bias_ap(t, j, span))
                Gs = {}
                for sc in phase:
                    for j in range(4):
                        ib = IBASE[j]
                        gb = GBASE[j]
                        G = gp.tile([64, CW], dt, tag="G", name=f"G_{ph}_{t}_{sc}_{j}")
                        Gs[(sc, j)] = G
                        nc.vector.tensor_scalar(G[ib:ib + w, :],
                                                Ss[(sc, j)][gb:gb + w, :],
                                                2.0, 1.0, ALU.mult, ALU.subtract)
                U4s = {}
                for sc in phase:
                    U4 = u4p.tile([128, CW], dt, tag="U4", name=f"U4_{ph}_{t}_{sc}")
                    U4s[sc] = U4
                    for j in range(4):
                        ib = IBASE[j]
                        ob = OBASE[j]
                        nc.vector.tensor_tensor(U4[ob:ob + w, :],
                                                Ss[(sc, j)][ib:ib + w, :],
                                                Gs[(sc, j)][ib:ib + w, :],
                                                ALU.mult)
                T4s = {}
                for sc in phase:
                    spanT = OBASE[3] + w
                    T4 = t4p.tile([128, CW], dt, tag="T4", name=f"T4_{ph}_{t}_{sc}")
                    T4s[sc] = T4
                    nc.scalar.activation(T4[0:spanT, :], U4s[sc][0:spanT, :], AF.Tanh)
                for sc in phase:
                    for j in range(4):
                        ob = OBASE[j]
                        nc.vector.tensor_tensor(
                            rhs[(t + 1, sc)][0:w, j * CW:(j + 1) * CW],
                            Ss[(sc, j)][ob:ob + w, :],
                            T4s[sc][ob:ob + w, :],
                            ALU.mult)
                for sc in phase:
                    for (src, n, dname, drow) in CONCAT[t]:
                        nc.gpsimd.dma_start(
                            out=hgt[(dname, sc)][drow:drow + n, :],
                            in_=rhs[(t + 1, sc)][src:src + n, :])
                    if t == 6:
                        nc.gpsimd.dma_start(out=get_rhs(8, sc)[30:40, :],
                                          in_=rhs[(7, sc)][20:30, :])

            def head_thunks(ph, phase, hgt):
                thunks = []
                F1s = {}
                Fos = {}

                def p1f1(sc, c):
                    def f():
                        p1 = pg.tile([128, CW], f32, tag="pg", name=f"p1_{ph}_{sc}_{c}")
                        for m in range(CW // MW):
                            col = slice(c * CW + m * MW, c * CW + (m + 1) * MW)
                            nc.tensor.matmul(p1[0:50, m * MW:(m + 1) * MW],
                                             hw_ap("w1T", 100, 50),
                                             hgt[("HG", sc)][0:100, col],
                                             start=True, stop=True)
                        F1 = f1p.tile([50, CW], dt, tag="F1", name=f"F1_{ph}_{sc}_{c}")
                        F1s[(sc, c)] = F1
                        nc.scalar.activation(F1[0:50, :], p1[0:50, :], AF.Tanh,
                                             bias=hb_ap(50, 0))
                    return f

                def pofo(sc, c):
                    def f():
                        po = pg.tile([128, CW], f32, tag="pg", name=f"po_{ph}_{sc}_{c}")
                        for m in range(CW // MW):
                            col = slice(c * CW + m * MW, c * CW + (m + 1) * MW)
                            mcol = slice(m * MW, (m + 1) * MW)
                            nc.tensor.matmul(po[0:100, mcol], hw_ap("poA", 100, 100),
                                             hgt[("HOa", sc)][0:100, col],
                                             start=True, stop=False)
                            nc.tensor.matmul(po[0:100, mcol], hw_ap("poB", 100, 100),
                                             hgt[("HOb", sc)][0:100, col],
                                             start=False, stop=True)
                        Fo = fop.tile([100, CW], dt, tag="Fo", name=f"Fo_{ph}_{sc}_{c}")
                        Fos[(sc, c)] = Fo
                        nc.scalar.activation(Fo[0:100, :], po[0:100, :], AF.Tanh,
                                             bias=hb_ap(100, 1))
                    return f

                def grp(gi, members):
                    def f():
                        p2 = pg.tile([128, CW], f32, tag="pg", name=f"p2_{ph}_{gi}")
                        for i, (sc, c) in enumerate(members):
                            orow = 32 * i
                            for m in range(CW // MW):
                                mcol = slice(m * MW, (m + 1) * MW)
                                nc.tensor.matmul(p2[orow:orow + 10, mcol],
                                                 hw_ap("w2aT", 50, 10),
                                                 F1s[(sc, c)][0:50, mcol],
                                                 start=True, stop=False)
                                nc.tensor.matmul(p2[orow:orow + 10, mcol],
                                                 hw_ap("w2bT", 100, 10),
                                                 Fos[(sc, c)][0:100, mcol],
                                                 start=False, stop=True)
                        gs = 32 * (len(members) - 1) + 10
                        F2 = f2p.tile([74, CW], dt, tag="F2", name=f"F2_{ph}_{gi}")
                        nc.scalar.activation(F2[0:gs, :], p2[0:gs, :], AF.Tanh,
                                             bias=hb_ap(gs, 2))
                        p3 = pg.tile([128, CW], f32, tag="pg", name=f"p3_{ph}_{gi}")
                        w3o = coff["w3rep"]
                        for i in range(len(members)):
                            orow = 32 * i
                            for m in range(CW // MW):
                                mcol = slice(m * MW, (m + 1) * MW)
                                nc.tensor.matmul(p3[orow:orow + 1, mcol],
                                                 CB[orow:orow + 10, w3o:w3o + 1],
                                                 F2[orow:orow + 10, mcol],
                                                 start=True, stop=True)
                        os_ = 32 * (len(members) - 1) + 1
                        ot = outp.tile([65, CW], f32, tag="out", name=f"ot_{ph}_{gi}")
                        nc.scalar.activation(ot[0:os_, :], p3[0:os_, :], AF.Tanh,
                                             bias=hb_ap(os_, 3))
                        row0 = ph * 8 + 3 * gi
                        nrow = len(members)
                        nc.sync.dma_start(out=out_d[row0:row0 + nrow, :],
                                          in_=ot[0:os_:32, :])
                    return f

                locals_ = [(sc, c) for sc in phase for c in range(4)]
                for sc, c in locals_:
                    thunks.append(p1f1(sc, c))
                for sc, c in locals_:
                    thunks.append(pofo(sc, c))
                groups = [locals_[i:i + 3] for i in range(0, 8, 3)]
                for gi, members in enumerate(groups):
                    thunks.append(grp(gi, members))
                return thunks

            pending = []
            front = []
            for ph, phase in enumerate(PHASES):
                rhs = {}
                hgt = {}
                for f in front:
                    f()
                front = []
                for sc in phase:
                    hgt[("HG", sc)] = hcat.tile([100, TW], dt, tag="HG", name=f"HG_{ph}_{sc}")
                    hgt[("HOa", sc)] = hcat.tile([100, TW], dt, tag="HOa", name=f"HOa_{ph}_{sc}")
                    hgt[("HOb", sc)] = hcat.tile([100, TW], dt, tag="HOb", name=f"HOb_{ph}_{sc}")
                    dst, src, n = SLOT_XDMA[0]
                    rhs[(0, sc)] = rhsp.tile([RHS_ROWS[0], TW], dt, tag="rhs",
                                             name=f"rhs_{ph}_0_{sc}")
                    nc.sync.dma_start(out=rhs[(0, sc)][dst:dst + n, :],
                                      in_=dr["xT32"][src:src + n, sc * TW:(sc + 1) * TW])
                last = ph == len(PHASES) - 1
                for t in range(NSLOT):
                    emit_slot(ph, phase, t, rhs, hgt)
                    # drain prior phase's head work between slots
                    ndrain = 2 if t < 4 else 1
                    for _ in range(ndrain):
                        if pending:
                            pending.pop(0)()
                    if last and t == NSLOT - 2:
                        # HG complete after slot 9: emit last phase's p1/F1 now
                        thunks = head_thunks(ph, phase, hgt)
                        for f in thunks[:8]:
                            f()
                        thunks = thunks[8:]
                if not last:
                    thunks = head_thunks(ph, phase, hgt)
                if INTERLEAVE_HEADS and not last:
                    front = thunks[:8]            # p1/F1 at next phase start
                    pending = pending + thunks[8:]  # po/Fo + groups interleave
                else:
                    for f in pending:
                        f()
                    pending = []
                    for f in thunks:
                        f()

    nc.finalize()
    return nc


def kernel(**inputs):
    import ml_dtypes
    np_dt = ml_dtypes.bfloat16
    inputs = {k: np.asarray(v) for k, v in inputs.items()}
    zero_state = not (inputs["gen_h"].any() or inputs["gen_c"].any()
                     or inputs["opp_h"].any() or inputs["opp_c"].any())
    from concourse.bass_utils import run_bass_kernel_spmd
    if zero_state:
        packed = pack_fast(inputs, np_dt)
        nc = build_fast(BC, np_dt)
        in_maps = []
        for c in range(NCORE):
            m = {}
            for k, v in packed.items():
                if k == "xT32":
                    m[k] = np.ascontiguousarray(v[:, c * BC:(c + 1) * BC])
                else:
                    m[k] = v
            in_maps.append(m)
        res = run_bass_kernel_spmd(nc, in_maps, list(range(NCORE)))
        outs = [res.results[c]["out16"].reshape(-1) for c in range(NCORE)]
        return np.concatenate(outs).reshape(B, 1).astype(np.float32)

    # general fallback (nonzero states): original slot kernel
    FD = 2048
    packed = pack_host(inputs, np_dt)
    nc = build_nc(BC, FD, np_dt)
    batch_keys = [k for k in packed if k.startswith(("st", "ct"))]
    in_maps = []
    for c in range(NCORE):
        m = {}
        for k, v in packed.items():
            if k in batch_keys:
                m[k] = np.ascontiguousarray(v[:, c * BC:(c + 1) * BC])
            else:
                m[k] = v
        in_maps.append(m)
    res = run_bass_kernel_spmd(nc, in_maps, list(range(NCORE)))
    outs = [res.results[c]["out"].reshape(-1) for c in range(NCORE)]
    return np.concatenate(outs).reshape(B, 1).astype(np.float32)


if __name__ == "__main__":
    pass


# revision 22
# speedup vs baseline: 3.3964x; 1.0036x over previous
"""Trainium2 Bass kernel for nn_Net_6maxFull (batch of tiny LSTM chains).

Strategy (pure data parallel over 8 cores, batch on the free axis):
  - 30 LSTM cells scheduled into 12 "slots" of up to 3 independent cells.
  - Per slot one block-diagonal matmul computes all gates:
      lhsT [K, M] host-packed: K = [h-chain rows | x/state rows],
      M = gate rows grouped 32-aligned: i@0, f@32, o@64, g@96.
  - Biases folded into ScalarE activation bias APs.
  - sigmoid(i,f,o) in one ACT instr, tanh(g) in one, c2-tanh in one.
  - c2 = f*c + i*g via two DVE ops using 32-aligned partition groups.
  - h written straight into the next slot's matmul rhs tile; copies of h
    into head-concat tiles go over SBUF->SBUF DMA.
  - Heads (W1/W1o/W2/W3) as small matmuls at end of each batch tile.
"""
import sys
import numpy as np

sys.path.insert(0, "/opt/trn_rl_repo")

B = 131072
NCORE = 8
BC = B // NCORE
H = 10

# slot schedule: list of cells; cell = ("g", layer) or ("o", branch, step)
SLOTS_G = (
    [[("g", 0), ("o", 0, 0), ("o", 1, 0)],
     [("g", 1), ("o", 0, 1), ("o", 1, 1)],
     [("g", 2), ("o", 0, 2), ("o", 1, 2)],
     [("g", 3), ("o", 0, 3), ("o", 1, 3)],
     [("g", 4), ("o", 2, 0), ("o", 3, 0)],
     [("g", 5), ("o", 2, 1), ("o", 3, 1)],
     [("g", 6), ("o", 2, 2), ("o", 3, 2)],
     [("g", 7), ("o", 2, 3), ("o", 3, 3)],
     [("g", 8), ("o", 4, 0)],
     [("g", 9), ("o", 4, 1)],
     [("o", 4, 2)],
     [("o", 4, 3)]]
)
NSLOT_G = len(SLOTS_G)

# gate group partition offsets inside the gates psum/sbuf tiles
GI, GF, GO, GG = 0, 32, 64, 96


def _is_start(cell):
    return (cell[0] == "g" and cell[1] == 0) or (cell[0] == "o" and cell[2] == 0)


def _pred(cell):
    if cell[0] == "g":
        return ("g", cell[1] - 1)
    return ("o", cell[1], cell[2] - 1)


def _x_rows(cell):
    # row range of x^T feeding a chain-start cell
    if cell[0] == "g":
        return (0, 12)
    p = cell[1]
    s = 12 + 5 * p + 1
    return (s, s + 4)


class Plan:
    """Host-side packing plan: row layouts of st/ct blocks and lhsT maps."""

    def __init__(self):
        self.slot = []
        for t, cells in enumerate(SLOTS_G):
            info = {"cells": cells, "nc": len(cells)}
            info["hp"] = 0 if t == 0 else 10 * len(SLOTS_G[t - 1])
            # DMA block rows: x rows for start cells then h-state rows per cell
            rows = []  # list of (kind, cell) kind in {x, h}
            for c in cells:
                if _is_start(c):
                    rows.append(("x", c))
            for c in cells:
                rows.append(("h", c))
            info["strows"] = rows
            info["R"] = sum(4 if (k == "x" and c[0] == "o") else
                            12 if (k == "x") else 10 for k, c in rows)
            info["K"] = info["hp"] + info["R"]
            info["M"] = 128  # g-group padded to full 32 rows
            self.slot.append(info)


PLAN = Plan()


def pack_host(inp, np_dt):
    """Build all DRAM-side arrays (full batch; sharding happens later).

    Returns dict name -> np.ndarray. Batch-carrying arrays have shape
    [rows, B]; weights/bias arrays are replicated across cores.
    """
    f32 = np.float32
    out = {}
    Bt = inp["x"].shape[0]
    xT = np.ascontiguousarray(inp["x"].T.astype(np_dt))            # [37, B]
    genh = {i: np.ascontiguousarray(inp["gen_h"][i].T.astype(np_dt)) for i in range(10)}
    genc = {i: np.ascontiguousarray(inp["gen_c"][i].T.astype(np_dt)) for i in range(10)}
    opph = {(p, s): np.ascontiguousarray(inp["opp_h"][p][s].T.astype(np_dt))
            for p in range(5) for s in range(4)}
    oppc = {(p, s): np.ascontiguousarray(inp["opp_c"][p][s].T.astype(np_dt))
            for p in range(5) for s in range(4)}

    def cell_w(cell):
        # returns Wih [40, din], Whh [40, 10], bias [40]
        if cell[0] == "g":
            i = cell[1]
            if i == 0:
                return (inp["W_g0_ih"], inp["W_g0_hh"],
                        inp["b_g0_ih"] + inp["b_g0_hh"])
            return (inp["W_g_ih"][i - 1], inp["W_g_hh"][i - 1],
                    inp["b_g_ih"][i - 1] + inp["b_g_hh"][i - 1])
        p, s = cell[1], cell[2]
        if s == 0:
            return (inp["W_o0_ih"][p], inp["W_o0_hh"][p],
                    inp["b_o0_ih"][p] + inp["b_o0_hh"][p])
        return (inp["W_o_ih"][p][s - 1], inp["W_o_hh"][p][s - 1],
                inp["b_o_ih"][p][s - 1] + inp["b_o_hh"][p][s - 1])

    for t, info in enumerate(PLAN.slot):
        cells = info["cells"]
        # ---- st block [R, B] ----
        st = np.empty((info["R"], Bt), np_dt)
        row_of = {}
        r = 0
        for kind, c in info["strows"]:
            if kind == "x":
                a, b = _x_rows(c)
                st[r:r + (b - a)] = xT[a:b]
                row_of[("x", c)] = r
                r += b - a
            else:
                src = genh[c[1]] if c[0] == "g" else opph[(c[1], c[2])]
                st[r:r + 10] = src
                row_of[("h", c)] = r
                r += 10
        out[f"st{t}"] = st
        # ---- ct block [10*nc, B] ----
        ct = np.empty((10 * info["nc"], Bt), np_dt)
        for k, c in enumerate(cells):
            src = genc[c[1]] if c[0] == "g" else oppc[(c[1], c[2])]
            ct[10 * k:10 * k + 10] = src
        out[f"ct{t}"] = ct
        # ---- lhsT [K, M] and bias [128] ----
        lw = np.zeros((info["K"], info["M"]), f32)
        bias = np.zeros((128, 1), f32)
        prev_cells = SLOTS_G[t - 1] if t > 0 else []
        for k, c in enumerate(cells):
            Wih, Whh, bvec = cell_w(c)
            Wih = np.asarray(Wih, f32)
            Whh = np.asarray(Whh, f32)
            bvec = np.asarray(bvec, f32)
            # gate row slices in torch order i,f,g,o
            gslice = {"i": slice(0, 10), "f": slice(10, 20),
                      "g": slice(20, 30), "o": slice(30, 40)}
            goff = {"i": GI + 10 * k, "f": GF + 10 * k,
                    "o": GO + 10 * k, "g": GG + 10 * k}
            # tanh-everywhere: sigmoid(x) = (tanh(x/2)+1)/2, so pre-acts of
            # i,f,o are halved; chained h inputs carry h' = 2h, so those
            # columns get an extra 0.5.
            gsc = {"i": 0.5, "f": 0.5, "o": 0.5, "g": 1.0}
            if _is_start(c):
                r0 = info["hp"] + row_of[("x", c)]
                din = Wih.shape[1]
                for gn in "ifog":
                    lw[r0:r0 + din, goff[gn]:goff[gn] + 10] = gsc[gn] * Wih[gslice[gn]].T
            else:
                pos = prev_cells.index(_pred(c))
                r0 = 10 * pos
                for gn in "ifog":
                    lw[r0:r0 + 10, goff[gn]:goff[gn] + 10] = 0.5 * gsc[gn] * Wih[gslice[gn]].T
            # state rows
            r0 = info["hp"] + row_of[("h", c)]
            for gn in "ifog":
                lw[r0:r0 + 10, goff[gn]:goff[gn] + 10] = gsc[gn] * Whh[gslice[gn]].T
                bias[goff[gn]:goff[gn] + 10, 0] = gsc[gn] * bvec[gslice[gn]]
        out[f"lw{t}"] = lw.astype(np_dt)
        out[f"bias{t}"] = bias

    # ---- heads ----
    W1 = np.asarray(inp["W1"], f32)      # [50, 100]
    W1o = np.asarray(inp["W1o"], f32)    # [20, 40]
    W2 = np.asarray(inp["W2"], f32)      # [10, 70]
    W3 = np.asarray(inp["W3"], f32)      # [1, 10]
    out["whg"] = (0.5 * W1.T).copy().astype(np_dt)       # [100, 50]
    who = np.zeros((80, 40), f32)
    for s in range(4):
        blk = 0.5 * W1o[:, 10 * s:10 * s + 10].T         # [10, 20]
        who[20 * s:20 * s + 10, 0:20] = blk
        who[20 * s + 10:20 * s + 20, 20:40] = blk
    out["who01"] = who.astype(np_dt)
    out["who23"] = who.astype(np_dt)
    out["who4"] = (0.5 * W1o.T).copy().astype(np_dt)     # [40, 20]
    out["w2a"] = W2[:, 0:50].T.copy().astype(np_dt)      # [50, 10]
    w2o = (W2[:, 50:70] / 5.0).T                          # [20, 10]
    out["w2b"] = np.vstack([w2o, w2o]).astype(np_dt)     # [40, 10]
    out["w2c"] = w2o.copy().astype(np_dt)                # [20, 10]
    out["w3"] = W3.T.copy().astype(np_dt)                # [10, 1]
    for w_ in (10, 20, 30):
        ia = np.zeros((GF + w_, w_), f32)
        for r in range(w_):
            ia[r, r] = 0.5
            ia[GF + r, r] = 0.5
        out[f"iadd{w_}"] = ia.astype(np_dt)
    hb = np.zeros((128, 8), f32)
    hb[0:50, 0] = np.asarray(inp["b1"], f32)
    hb[0:40, 1] = np.tile(np.asarray(inp["b1o"], f32), 2)
    hb[0:20, 2] = np.asarray(inp["b1o"], f32)
    hb[0:10, 3] = np.asarray(inp["b2"], f32)
    hb[0:1, 4] = np.asarray(inp["b3"], f32)
    out["hbias"] = hb
    return out


def build_nc(Bc, FD, np_dt):
    """Build the SPMD Bass program for one core over Bc batch columns."""
    import concourse.bass as bass
    import concourse.tile as tile
    from concourse import bacc, mybir

    dt = {np.dtype(np.float32): mybir.dt.float32}.get(np.dtype(np_dt))
    if dt is None:
        import ml_dtypes
        assert np.dtype(np_dt) == np.dtype(ml_dtypes.bfloat16)
        dt = mybir.dt.bfloat16
    f32 = mybir.dt.float32
    AF = mybir.ActivationFunctionType

    PSUM_FD = min(1024, FD)
    N_MM = min(512, PSUM_FD)
    n_tiles = Bc // FD
    assert Bc % FD == 0 and FD % PSUM_FD == 0 and PSUM_FD % N_MM == 0

    nc = bacc.Bacc(None, target_bir_lowering=False, debug=False)
    P = PLAN.slot
    dr = {}
    for t in range(NSLOT_G):
        dr[f"st{t}"] = nc.declare_dram_parameter(f"st{t}", [P[t]["R"], Bc], dt, isOutput=False)
        dr[f"ct{t}"] = nc.declare_dram_parameter(f"ct{t}", [10 * P[t]["nc"], Bc], dt, isOutput=False)
        dr[f"lw{t}"] = nc.declare_dram_parameter(f"lw{t}", [P[t]["K"], P[t]["M"]], dt, isOutput=False)
        dr[f"bias{t}"] = nc.declare_dram_parameter(f"bias{t}", [128, 1], f32, isOutput=False)
    for name, shp in [("whg", [100, 50]), ("who01", [80, 40]), ("who23", [80, 40]),
                      ("who4", [40, 20]), ("w2a", [50, 10]), ("w2b", [40, 10]),
                      ("w2c", [20, 10]), ("w3", [10, 1]),
                      ("iadd10", [42, 10]), ("iadd20", [52, 20]), ("iadd30", [62, 30])]:
        dr[name] = nc.declare_dram_parameter(name, shp, dt, isOutput=False)
    dr["hbias"] = nc.declare_dram_parameter("hbias", [128, 8], f32, isOutput=False)
    out_d = nc.declare_dram_parameter("out", [1, Bc], f32, isOutput=True)

    from contextlib import ExitStack
    with tile.TileContext(nc) as tc:
        with ExitStack() as ctx:
            consts = ctx.enter_context(tc.tile_pool(name="consts", bufs=1))
            rhsp = ctx.enter_context(tc.tile_pool(name="rhs", bufs=7))
            sp = ctx.enter_context(tc.tile_pool(name="sig", bufs=4))
            zp = ctx.enter_context(tc.tile_pool(name="z", bufs=4))
            up = ctx.enter_context(tc.tile_pool(name="u", bufs=3))
            cp = ctx.enter_context(tc.tile_pool(name="c2", bufs=3))
            hp_ = ctx.enter_context(tc.tile_pool(name="hcat", bufs=2))
            fp = ctx.enter_context(tc.tile_pool(name="fh", bufs=1))
            outp = ctx.enter_context(tc.tile_pool(name="outp", bufs=2))
            pg = ctx.enter_context(tc.tile_pool(name="pgate", bufs=2, space="PSUM"))

            # ---- constants ----
            lw = {}
            bias = {}
            for t in range(NSLOT_G):
                lw[t] = consts.tile([P[t]["K"], P[t]["M"]], dt, tag=f"lw{t}", name=f"lw{t}")
                nc.sync.dma_start(out=lw[t], in_=dr[f"lw{t}"][:])
                bias[t] = consts.tile([128, 1], f32, tag=f"bias{t}", name=f"biast{t}")
                nc.sync.dma_start(out=bias[t], in_=dr[f"bias{t}"][:])
            hw = {}
            for name in ["whg", "who01", "who23", "who4", "w2a", "w2b", "w2c", "w3",
                         "iadd10", "iadd20", "iadd30"]:
                hw[name] = consts.tile(list(dr[name].shape), dt, tag=name, name=f"hw_{name}")
                nc.sync.dma_start(out=hw[name], in_=dr[name][:])
            hb = consts.tile([128, 8], f32, tag="hbias")
            nc.sync.dma_start(out=hb, in_=dr["hbias"][:])


            for it in range(n_tiles):
                col = slice(it * FD, (it + 1) * FD)
                # head concat tiles
                HG = hp_.tile([100, FD], dt, tag="HG")
                HO = {0: hp_.tile([80, FD], dt, tag="HO01", name=f"HO01_{it}"),
                      1: hp_.tile([80, FD], dt, tag="HO23", name=f"HO23_{it}"),
                      2: hp_.tile([40, FD], dt, tag="HO4", name=f"HO4_{it}")}
                rhs = {}
                for t in range(NSLOT_G + 1):
                    kt = P[t]["K"] if t < NSLOT_G else 10
                    rhs[t] = rhsp.tile([kt, FD], dt, tag="rhs", name=f"rhs_{it}_{t}")
                # stage first two state DMAs; the rest issue inside the loop
                for t in (0, 1):
                    nc.sync.dma_start(out=rhs[t][P[t]["hp"]:P[t]["K"], :],
                                      in_=dr[f"st{t}"][:, col])

                for t in range(NSLOT_G):
                    info = P[t]
                    ncell = info["nc"]
                    w = 10 * ncell
                    if t + 2 < NSLOT_G:
                        t2 = t + 2
                        nc.sync.dma_start(out=rhs[t2][P[t2]["hp"]:P[t2]["K"], :],
                                          in_=dr[f"st{t2}"][:, col])
                    S = sp.tile([128, FD], dt, tag="S", name=f"S_{it}_{t}")
                    Z = zp.tile([GF + 32, FD], dt, tag="Z", name=f"Z_{it}_{t}")
                    U = up.tile([GF + 32, FD], dt, tag="U", name=f"U_{it}_{t}")
                    T2 = cp.tile([GO + 32, FD], dt, tag="T2", name=f"T2_{it}_{t}")
                    # c states -> Z[32:32+w]
                    nc.sync.dma_start(out=Z[GF:GF + w, :], in_=dr[f"ct{t}"][:, col])
                    pt = pg.tile([128, FD], f32, tag="pg", name=f"pg_{it}_{t}")
                    for m in range(FD // N_MM):
                        mcol = slice(m * N_MM, (m + 1) * N_MM)
                        nc.tensor.matmul(pt[:, mcol], lw[t][:],
                                         rhs[t][0:info["K"], mcol],
                                         start=True, stop=True)
                    # tanh over ALL gate groups (i,f,o pre-halved on host)
                    nc.scalar.activation(S[0:128, :], pt[0:128, :],
                                         AF.Tanh, bias=bias[t][0:128])
                    # move tanh(g) next to c for the fused product
                    nc.vector.tensor_copy(Z[0:32, :], S[GG:GG + 32, :])
                    # U = (T_if + 1) * [g | c]
                    nc.vector.scalar_tensor_tensor(
                        U[0:GF + w], S[0:GF + w], 1.0, Z[0:GF + w],
                        mybir.AluOpType.add, mybir.AluOpType.mult)
                    # c2 = 0.5*(row + row+32) back into pt[0:w] (psum reuse)
                    iw = hw[f"iadd{w}"]
                    for m in range(FD // N_MM):
                        mcol = slice(m * N_MM, (m + 1) * N_MM)
                        nc.tensor.matmul(pt[0:w, mcol], iw[:],
                                         U[0:GF + w, mcol],
                                         start=True, stop=True)
                    # T2 = tanh(c2) at base GO (pairs with T_o)
                    nc.scalar.activation(T2[GO:GO + w, :], pt[0:w, :], AF.Tanh)
                    # h' = 2h = (T_o + 1) * tanh(c2) -> next slot rhs rows 0:w
                    nc.vector.scalar_tensor_tensor(
                        rhs[t + 1][0:w, :], S[GO:GO + w, :], 1.0, T2[GO:GO + w, :],
                        mybir.AluOpType.add, mybir.AluOpType.mult)
                    # copy h pieces into head concat tiles (SBUF->SBUF DMA)
                    hsrc = rhs[t + 1]
                    if SLOTS_G[t][0][0] == "g":
                        gi = SLOTS_G[t][0][1]
                        nc.sync.dma_start(out=HG[10 * gi:10 * gi + 10, :], in_=hsrc[0:10, :])
                    for k, c in enumerate(SLOTS_G[t]):
                        if c[0] == "o":
                            p, s = c[1], c[2]
                            pair = p // 2 if p < 4 else 2
                            drow = (20 * s + 10 * (p % 2)) if p < 4 else 10 * s
                            nc.sync.dma_start(out=HO[pair][drow:drow + 10, :],
                                              in_=hsrc[10 * k:10 * k + 10, :])

                # ---- heads ----
                F1 = fp.tile([50, FD], dt, tag="F1", name=f"F1_{it}")
                Fo = {0: fp.tile([40, FD], dt, tag="Fo01", name=f"Fo01_{it}"),
                      1: fp.tile([40, FD], dt, tag="Fo23", name=f"Fo23_{it}"),
                      2: fp.tile([20, FD], dt, tag="Fo4", name=f"Fo4_{it}")}
                F2 = fp.tile([10, FD], dt, tag="F2", name=f"F2_{it}")
                out_sb = outp.tile([1, FD], f32, tag="out", name=f"out_{it}")

                def head_mm(psname, pairs, nrow, bias_ap, Fdst):
                    p_ = pg.tile([128, FD], f32, tag="pg", name=psname)
                    for m in range(FD // N_MM):
                        mc = slice(m * N_MM, (m + 1) * N_MM)
                        for j, (lh, rh) in enumerate(pairs):
                            nc.tensor.matmul(p_[0:nrow, mc], lh[:], rh[:, mc],
                                             start=(j == 0), stop=(j == len(pairs) - 1))
                    nc.scalar.activation(Fdst[0:nrow, :], p_[0:nrow, :],
                                         AF.Tanh, bias=bias_ap)

                head_mm(f"p1_{it}", [(hw["whg"], HG)], 50, hb[0:50, 0:1], F1)
                head_mm(f"po1_{it}", [(hw["who01"], HO[0])], 40, hb[0:40, 1:2], Fo[0])
                head_mm(f"po2_{it}", [(hw["who23"], HO[1])], 40, hb[0:40, 1:2], Fo[1])
                head_mm(f"po3_{it}", [(hw["who4"], HO[2])], 20, hb[0:20, 2:3], Fo[2])
                head_mm(f"p2_{it}",
                        [(hw["w2a"], F1), (hw["w2b"], Fo[0]),
                         (hw["w2b"], Fo[1]), (hw["w2c"], Fo[2])],
                        10, hb[0:10, 3:4], F2)
                p3 = pg.tile([128, FD], f32, tag="pg", name=f"p3_{it}")
                for m in range(FD // N_MM):
                    mc = slice(m * N_MM, (m + 1) * N_MM)
                    nc.tensor.matmul(p3[0:1, mc], hw["w3"][:], F2[:, mc],
                                     start=True, stop=True)
                nc.scalar.activation(out_sb[0:1, :], p3[0:1, :],
                                     AF.Tanh, bias=hb[0:1, 4:5])
                nc.sync.dma_start(out=out_d[0:1, col], in_=out_sb)

    nc.finalize()
    return nc




# ============ fast path (zero initial states) ============

TW = 4096            # tile width (super-chunk)
CW = 1024            # compute width (sub-chunk)
MW = 512             # matmul moving width
NSC = BC // TW       # 4 super-chunks
PHASES = [[0, 1], [2, 3]]
INTERLEAVE_HEADS = True

# rotation bases per sub-chunk j (o-group must sit at 32*j)
OBASE = [0, 32, 64, 96]
IBASE = [32, 0, 0, 0]
GBASE = [64, 64, 32, 32]

# cells: ("g", layer) or ("o", branch, step)
SLOT_CELLS = [
    [("g", 0), ("o", 0, 0), ("o", 1, 0)],
    [("g", 1), ("o", 0, 1), ("o", 1, 1)],
    [("g", 2), ("o", 0, 2), ("o", 1, 2)],
    [("g", 3), ("o", 0, 3), ("o", 1, 3)],
    [("g", 4), ("o", 2, 0), ("o", 3, 0)],
    [("g", 5), ("o", 2, 1), ("o", 3, 1)],
    [("g", 6), ("o", 2, 2), ("o", 3, 2)],
    [("g", 7), ("o", 2, 3), ("o", 4, 0)],
    [("g", 8), ("o", 3, 3), ("o", 4, 1)],
    [("o", 4, 2), ("g", 9)],
    [("o", 4, 3)],
]
NSLOT = 11
# per-slot: input row range (start,len) in rhs[t] for each cell
SLOT_INROWS = [
    [(0, 12), (12, 4), (16, 4)],
    [(0, 10), (10, 10), (20, 10)],
    [(0, 10), (10, 10), (20, 10)],
    [(0, 10), (10, 10), (20, 10)],
    [(0, 10), (30, 4), (34, 4)],
    [(0, 10), (10, 10), (20, 10)],
    [(0, 10), (10, 10), (20, 10)],
    [(0, 10), (10, 10), (30, 4)],
    [(0, 10), (30, 10), (20, 10)],
    [(20, 10), (0, 10)],
    [(0, 10)],
]
SLOT_K = [20, 30, 30, 30, 38, 30, 30, 34, 40, 30, 10]
RHS_ROWS = [20, 30, 30, 30, 38, 30, 30, 34, 40, 30, 20, 10]
# x DMA per slot: (dst_row_in_rhs, src_row_in_xT32, nrows)
SLOT_XDMA = {0: (0, 0, 20), 4: (30, 20, 8), 7: (30, 28, 4)}
# concat: per slot list of (src_row, n, dst_name, dst_row)
CONCAT = [
    [(0, 10, "HG", 0), (10, 10, "HOa", 0), (20, 10, "HOa", 40)],
    [(0, 10, "HG", 10), (10, 10, "HOa", 10), (20, 10, "HOa", 50)],
    [(0, 10, "HG", 20), (10, 10, "HOa", 20), (20, 10, "HOa", 60)],
    [(0, 10, "HG", 30), (10, 10, "HOa", 30), (20, 10, "HOa", 70)],
    [(0, 10, "HG", 40), (10, 10, "HOb", 0), (20, 10, "HOb", 40)],
    [(0, 10, "HG", 50), (10, 10, "HOb", 10), (20, 10, "HOb", 50)],
    [(0, 10, "HG", 60), (10, 10, "HOb", 20), (20, 10, "HOb", 60)],
    [(0, 10, "HG", 70), (10, 10, "HOb", 30), (20, 10, "HOa", 80)],
    [(0, 10, "HG", 80), (10, 10, "HOb", 70), (20, 10, "HOa", 90)],
    [(0, 10, "HOb", 80), (10, 10, "HG", 90)],
    [(0, 10, "HOb", 90)],
]


def _w(t):
    return 10 * len(SLOT_CELLS[t])


def _span(t, j):
    return max(IBASE[j], OBASE[j], GBASE[j]) + _w(t)


def _cell_w(inp, cell):
    f32 = np.float32
    if cell[0] == "g":
        i = cell[1]
        if i == 0:
            W = np.asarray(inp["W_g0_ih"], f32)
            b = np.asarray(inp["b_g0_ih"], f32) + np.asarray(inp["b_g0_hh"], f32)
        else:
            W = np.asarray(inp["W_g_ih"][i - 1], f32)
            b = np.asarray(inp["b_g_ih"][i - 1], f32) + np.asarray(inp["b_g_hh"][i - 1], f32)
    else:
        p, s = cell[1], cell[2]
        if s == 0:
            W = np.asarray(inp["W_o0_ih"][p], f32)
            b = np.asarray(inp["b_o0_ih"][p], f32) + np.asarray(inp["b_o0_hh"][p], f32)
        else:
            W = np.asarray(inp["W_o_ih"][p][s - 1], f32)
            b = np.asarray(inp["b_o_ih"][p][s - 1], f32) + np.asarray(inp["b_o_hh"][p][s - 1], f32)
    return W, b


def pack_fast(inp, np_dt):
    """Host-side constant packing. Returns dict name -> np.ndarray.
    xT32 is [32, B] (batch-carrying, split per core later)."""
    f32 = np.float32
    out = {}
    x = np.asarray(inp["x"], f32)
    # rows: gen 0:12, then branch p active-stripped cols (4 each)
    xT = np.empty((32, x.shape[0]), f32)
    xT[0:12] = x[:, 0:12].T
    for p in range(5):
        s = 12 + 5 * p + 1
        xT[12 + 4 * p: 16 + 4 * p] = x[:, s:s + 4].T
    out["xT32"] = xT.astype(np_dt)

    for t in range(NSLOT):
        cells = SLOT_CELLS[t]
        w = _w(t)
        K = SLOT_K[t]
        bias = np.zeros((128, 4), f32)
        for j in range(4):
            lw = np.zeros((K, _span(t, j)), f32)
            for k, cell in enumerate(cells):
                W, b = _cell_w(inp, cell)
                r0, rn = SLOT_INROWS[t][k]
                Wi, Wg, Wo = W[0:10], W[20:30], W[30:40]
                bi, bg, bo = b[0:10], b[20:30], b[30:40]
                lw[r0:r0 + rn, IBASE[j] + 10 * k: IBASE[j] + 10 * k + 10] = Wi[:, 0:rn].T
                lw[r0:r0 + rn, OBASE[j] + 10 * k: OBASE[j] + 10 * k + 10] = Wo[:, 0:rn].T
                lw[r0:r0 + rn, GBASE[j] + 10 * k: GBASE[j] + 10 * k + 10] = 2.0 * Wg[:, 0:rn].T
                bias[IBASE[j] + 10 * k: IBASE[j] + 10 * k + 10, j] = bi
                bias[OBASE[j] + 10 * k: OBASE[j] + 10 * k + 10, j] = bo
                bias[GBASE[j] + 10 * k: GBASE[j] + 10 * k + 10, j] = 2.0 * bg
            out[f"lw{t}_{j}"] = lw.astype(np_dt)
        out[f"bias{t}"] = bias

    W1 = np.asarray(inp["W1"], f32)      # [50, 100]
    W1o = np.asarray(inp["W1o"], f32)    # [20, 40]
    W2 = np.asarray(inp["W2"], f32)      # [10, 70]
    W3 = np.asarray(inp["W3"], f32)      # [1, 10]
    out["w1T"] = W1.T.copy().astype(np_dt)               # [100, 50]
    poA = np.zeros((100, 100), f32)
    poB = np.zeros((100, 100), f32)
    poA[0:40, 0:20] = W1o.T
    poA[40:80, 20:40] = W1o.T
    poA[80:100, 80:100] = W1o[:, 0:20].T
    poB[0:40, 40:60] = W1o.T
    poB[40:80, 60:80] = W1o.T
    poB[80:100, 80:100] = W1o[:, 20:40].T
    out["poA"] = poA.astype(np_dt)
    out["poB"] = poB.astype(np_dt)
    out["w2aT"] = W2[:, 0:50].T.copy().astype(np_dt)     # [50, 10]
    w2b = (W2[:, 50:70] / 5.0).T                          # [20, 10]
    out["w2bT"] = np.tile(w2b, (5, 1)).astype(np_dt)     # [100, 10]
    w3rep = np.zeros((74, 1), f32)
    for i in range(3):
        w3rep[32 * i:32 * i + 10, 0] = W3[0]
    out["w3rep"] = w3rep.astype(np_dt)
    hb = np.zeros((128, 4), f32)
    hb[0:50, 0] = np.asarray(inp["b1"], f32)
    hb[0:100, 1] = np.tile(np.asarray(inp["b1o"], f32), 5)
    for i in range(3):
        hb[32 * i:32 * i + 10, 2] = np.asarray(inp["b2"], f32)
        hb[32 * i, 3] = np.asarray(inp["b3"], f32)[0]
    out["hbias"] = hb
    # ---- batch all constants into two arrays (2 DMAs at startup) ----
    coff, ncol = _const_layout()
    CB = np.zeros((128, ncol), np_dt)
    for t in range(NSLOT):
        for j in range(4):
            a = out.pop(f"lw{t}_{j}")
            CB[0:a.shape[0], coff[(t, j)]:coff[(t, j)] + a.shape[1]] = a
    for name in ["w1T", "poA", "poB", "w2aT", "w2bT", "w3rep"]:
        a = out.pop(name)
        CB[0:a.shape[0], coff[name]:coff[name] + a.shape[1]] = a
    CF = np.zeros((128, 48), f32)
    for t in range(NSLOT):
        CF[:, 4 * t:4 * t + 4] = out.pop(f"bias{t}")
    CF[:, 44:48] = out.pop("hbias")
    out["CB"] = CB
    out["CF"] = CF
    return out


def _const_layout():
    off = {}
    col = 0
    for t in range(NSLOT):
        for j in range(4):
            off[(t, j)] = col
            col += _span(t, j)
    for name, w_ in [("w1T", 50), ("poA", 100), ("poB", 100),
                     ("w2aT", 10), ("w2bT", 10), ("w3rep", 1)]:
        off[name] = col
        col += w_
    return off, col


def build_fast(Bc, np_dt):
    import concourse.tile as tile
    from concourse import bacc, mybir

    dt = mybir.dt.float32
    if np.dtype(np_dt) != np.dtype(np.float32):
        dt = mybir.dt.bfloat16
    f32 = mybir.dt.float32
    AF = mybir.ActivationFunctionType
    ALU = mybir.AluOpType

    assert Bc == NSC * TW
    coff, ncol = _const_layout()

    nc = bacc.Bacc(None, target_bir_lowering=False, debug=False)
    dr = {}
    dr["CB"] = nc.declare_dram_parameter("CB", [128, ncol], dt, isOutput=False)
    dr["CF"] = nc.declare_dram_parameter("CF", [128, 48], f32, isOutput=False)
    dr["xT32"] = nc.declare_dram_parameter("xT32", [32, Bc], dt, isOutput=False)
    out_d = nc.declare_dram_parameter("out16", [NSC * 4, CW], f32, isOutput=True)

    from contextlib import ExitStack
    with tile.TileContext(nc) as tc:
        with ExitStack() as ctx:
            consts = ctx.enter_context(tc.tile_pool(name="consts", bufs=1))
            rhsp = ctx.enter_context(tc.tile_pool(name="rhs", bufs=6))
            sp = ctx.enter_context(tc.tile_pool(name="S", bufs=8))
            gp = ctx.enter_context(tc.tile_pool(name="G", bufs=6))
            u4p = ctx.enter_context(tc.tile_pool(name="U4", bufs=3))
            t4p = ctx.enter_context(tc.tile_pool(name="T4", bufs=3))
            hcat = ctx.enter_context(tc.tile_pool(name="hcat", bufs=2))
            f1p = ctx.enter_context(tc.tile_pool(name="F1", bufs=8))
            fop = ctx.enter_context(tc.tile_pool(name="Fo", bufs=8))
            f2p = ctx.enter_context(tc.tile_pool(name="F2", bufs=3))
            outp = ctx.enter_context(tc.tile_pool(name="outp", bufs=2))
            pg = ctx.enter_context(tc.tile_pool(name="pg", bufs=4, space="PSUM"))

            CB = consts.tile([128, ncol], dt, tag="CB")
            nc.sync.dma_start(out=CB, in_=dr["CB"][:])
            CF = consts.tile([128, 48], f32, tag="CF")
            nc.sync.dma_start(out=CF, in_=dr["CF"][:])

            def lw_ap(t, j, K=None):
                o = coff[(t, j)]
                return CB[0:(K or SLOT_K[t]), o:o + _span(t, j)]

            def hw_ap(name, rows, colw):
                o = coff[name]
                return CB[0:rows, o:o + colw]

            def bias_ap(t, j, span):
                return CF[0:span, 4 * t + j:4 * t + j + 1]

            def hb_ap(rows, col):
                return CF[0:rows, 44 + col:44 + col + 1]

            state = {}

            def emit_slot(ph, phase, t, rhs, hgt):
                w = _w(t)
                K = SLOT_K[t]

                def get_rhs(t1, sc):
                    if (t1, sc) not in rhs:
                        rhs[(t1, sc)] = rhsp.tile([RHS_ROWS[t1], TW], dt, tag="rhs",
                                                  name=f"rhs_{ph}_{t1}_{sc}")
                    return rhs[(t1, sc)]

                for sc in phase:
                    get_rhs(t + 1, sc)
                    if (t + 1) in SLOT_XDMA:
                        dst, src, n = SLOT_XDMA[t + 1]
                        nc.sync.dma_start(
                            out=rhs[(t + 1, sc)][dst:dst + n, :],
                            in_=dr["xT32"][src:src + n, sc * TW:(sc + 1) * TW])
                pts = {}
                for j in range(4):
                    span = _span(t, j)
                    for sc in phase:
                        pt = pg.tile([128, CW], f32, tag="pg", name=f"pt_{ph}_{t}_{sc}_{j}")
                        pts[(sc, j)] = pt
                        for m in range(CW // MW):
                            col = slice(j * CW + m * MW, j * CW + (m + 1) * MW)
                            nc.tensor.matmul(pt[0:span, m * MW:(m + 1) * MW],
                                             lw_ap(t, j),
                                             rhs[(t, sc)][0:K, col],
                                             start=True, stop=True)
                Ss = {}
                for sc in phase:
                    for j in range(4):
                        span = _span(t, j)
                        S = sp.tile([128, CW], dt, tag="S", name=f"S_{ph}_{t}_{sc}_{j}")
                        Ss[(sc, j)] = S
                        nc.scalar.activation(S[0:span, :], pts[(sc, j)][0:span, :],
                                             AF.Sigmoid, bias=bias_ap(t, j, span))
                Gs = {}
                for sc in phase:
                    for j in range(4):
                        ib = IBASE[j]
                        gb = GBASE[j]
                        G = gp.tile([64, CW], dt, tag="G", name=f"G_{ph}_{t}_{sc}_{j}")
                        Gs[(sc, j)] = G
                        nc.vector.tensor_scalar(G[ib:ib + w, :],
                                                Ss[(sc, j)][gb:gb + w, :],
                                                2.0, 1.0, ALU.mult, ALU.subtract)
                U4s = {}
                for sc in phase:
                    U4 = u4p.tile([128, CW], dt, tag="U4", name=f"U4_{ph}_{t}_{sc}")
                    U4s[sc] = U4
                    for j in range(4):
                        ib = IBASE[j]
                        ob = OBASE[j]
                        nc.vector.tensor_tensor(U4[ob:ob + w, :],
                                                Ss[(sc, j)][ib:ib + w, :],
                                                Gs[(sc, j)][ib:ib + w, :],
                                                ALU.mult)
                T4s = {}
                for sc in phase:
                    spanT = OBASE[3] + w
                    T4 = t4p.tile([128, CW], dt, tag="T4", name=f"T4_{ph}_{t}_{sc}")
                    T4s[sc] = T4
                    nc.scalar.activation(T4[0:spanT, :], U4s[sc][0:spanT, :], AF.Tanh)
                for sc in phase:
                    for j in range(4):
                        ob = OBASE[j]
                        nc.vector.tensor_tensor(
                            rhs[(t + 1, sc)][0:w, j * CW:(j + 1) * CW],
                            Ss[(sc, j)][ob:ob + w, :],
                            T4s[sc][ob:ob + w, :],
                            ALU.mult)
                for sc in phase:
                    for (src, n, dname, drow) in CONCAT[t]:
                        nc.gpsimd.dma_start(
                            out=hgt[(dname, sc)][drow:drow + n, :],
                            in_=rhs[(t + 1, sc)][src:src + n, :])
                    if t == 6:
                        nc.gpsimd.dma_start(out=get_rhs(8, sc)[30:40, :],
                                          in_=rhs[(7, sc)][20:30, :])

            def head_thunks(ph, phase, hgt):
                thunks = []
                F1s = {}
                Fos = {}

                def p1f1(sc, c):
                    def f():
                        p1 = pg.tile([128, CW], f32, tag="pg", name=f"p1_{ph}_{sc}_{c}")
                        for m in range(CW // MW):
                            col = slice(c * CW + m * MW, c * CW + (m + 1) * MW)
                            nc.tensor.matmul(p1[0:50, m * MW:(m + 1) * MW],
                                             hw_ap("w1T", 100, 50),
                                             hgt[("HG", sc)][0:100, col],
                                             start=True, stop=True)
                        F1 = f1p.tile([50, CW], dt, tag="F1", name=f"F1_{ph}_{sc}_{c}")
                        F1s[(sc, c)] = F1
                        nc.scalar.activation(F1[0:50, :], p1[0:50, :], AF.Tanh,
                                             bias=hb_ap(50, 0))
                    return f

                def pofo(sc, c):
                    def f():
                        po = pg.tile([128, CW], f32, tag="pg", name=f"po_{ph}_{sc}_{c}")
                        for m in range(CW // MW):
                            col = slice(c * CW + m * MW, c * CW + (m + 1) * MW)
                            mcol = slice(m * MW, (m + 1) * MW)
                            nc.tensor.matmul(po[0:100, mcol], hw_ap("poA", 100, 100),
                                             hgt[("HOa", sc)][0:100, col],
                                             start=True, stop=False)
                            nc.tensor.matmul(po[0:100, mcol], hw_ap("poB", 100, 100),
                                             hgt[("HOb", sc)][0:100, col],
                                             start=False, stop=True)
                        Fo = fop.tile([100, CW], dt, tag="Fo", name=f"Fo_{ph}_{sc}_{c}")
                        Fos[(sc, c)] = Fo
                        nc.scalar.activation(Fo[0:100, :], po[0:100, :], AF.Tanh,
                                             bias=hb_ap(100, 1))
                    return f

                def grp(gi, members):
                    def f():
                        p2 = pg.tile([128, CW], f32, tag="pg", name=f"p2_{ph}_{gi}")
                        for i, (sc, c) in enumerate(members):
                            orow = 32 * i
                            for m in range(CW // MW):
                                mcol = slice(m * MW, (m + 1) * MW)
                                nc.tensor.matmul(p2[orow:orow + 10, mcol],
                                                 hw_ap("w2aT", 50, 10),
                                                 F1s[(sc, c)][0:50, mcol],
                                                 start=True, stop=False)
                                nc.tensor.matmul(p2[orow:orow + 10, mcol],
                                                 hw_ap("w2bT", 100, 10),
                                                 Fos[(sc, c)][0:100, mcol],
                                                 start=False, stop=True)
                        gs = 32 * (len(members) - 1) + 10
                        F2 = f2p.tile([74, CW], dt, tag="F2", name=f"F2_{ph}_{gi}")
                        nc.scalar.activation(F2[0:gs, :], p2[0:gs, :], AF.Tanh,
                                             bias=hb_ap(gs, 2))
                        p3 = pg.tile([128, CW], f32, tag="pg", name=f"p3_{ph}_{gi}")
                        w3o = coff["w3rep"]
                        for i in range(len(members)):
                            orow = 32 * i
                            for m in range(CW // MW):
                                mcol = slice(m * MW, (m + 1) * MW)
                                nc.tensor.matmul(p3[orow:orow + 1, mcol],
                                                 CB[orow:orow + 10, w3o:w3o + 1],
                                                 F2[orow:orow + 10, mcol],
                                                 start=True, stop=True)
                        os_ = 32 * (len(members) - 1) + 1
                        ot = outp.tile([65, CW], f32, tag="out", name=f"ot_{ph}_{gi}")
                        nc.scalar.activation(ot[0:os_, :], p3[0:os_, :], AF.Tanh,
                                             bias=hb_ap(os_, 3))
                        row0 = ph * 8 + 3 * gi
                        nrow = len(members)
                        nc.sync.dma_start(out=out_d[row0:row0 + nrow, :],
                                          in_=ot[0:os_:32, :])
                    return f

                locals_ = [(sc, c) for sc in phase for c in range(4)]
                for sc, c in locals_:
                    thunks.append(p1f1(sc, c))
                for sc, c in locals_:
                    thunks.append(pofo(sc, c))
                groups = [locals_[i:i + 3] for i in range(0, 8, 3)]
                for gi, members in enumerate(groups):
                    thunks.append(grp(gi, members))
                return thunks

            pending = []
            front = []
            for ph, phase in enumerate(PHASES):
                rhs = {}
                hgt = {}
                for f in front:
                    f()
                front = []
                for sc in phase:
                    hgt[("HG", sc)] = hcat.tile([100, TW], dt, tag="HG", name=f"HG_{ph}_{sc}")
                    hgt[("HOa", sc)] = hcat.tile([100, TW], dt, tag="HOa", name=f"HOa_{ph}_{sc}")
                    hgt[("HOb", sc)] = hcat.tile([100, TW], dt, tag="HOb", name=f"HOb_{ph}_{sc}")
                    dst, src, n = SLOT_XDMA[0]
                    rhs[(0, sc)] = rhsp.tile([RHS_ROWS[0], TW], dt, tag="rhs",
                                             name=f"rhs_{ph}_0_{sc}")
                    nc.sync.dma_start(out=rhs[(0, sc)][dst:dst + n, :],
                                      in_=dr["xT32"][src:src + n, sc * TW:(sc + 1) * TW])
                last = ph == len(PHASES) - 1
                for t in range(NSLOT):
                    emit_slot(ph, phase, t, rhs, hgt)
                    # drain prior phase's head work between slots
                    ndrain = 2 if t < 4 else 1
                    for _ in range(ndrain):
                        if pending:
                            pending.pop(0)()
                    if last and t == NSLOT - 2:
                        # HG complete after slot 9: emit last phase's p1/F1 now
                        thunks = head_thunks(ph, phase, hgt)
                        for f in thunks[:8]:
                            f()
                        thunks = thunks[8:]
                if not last:
                    thunks = head_thunks(ph, phase, hgt)
                if INTERLEAVE_HEADS and not last:
                    front = thunks[:8]            # p1/F1 at next phase start
                    pending = pending + thunks[8:]  # po/Fo + groups interleave
                else:
                    for f in pending:
                        f()
                    pending = []
                    for f in thunks:
                        f()

    nc.finalize()
    return nc


def kernel(**inputs):
    import ml_dtypes
    np_dt = ml_dtypes.bfloat16
    inputs = {k: np.asarray(v) for k, v in inputs.items()}
    zero_state = not (inputs["gen_h"].any() or inputs["gen_c"].any()
                     or inputs["opp_h"].any() or inputs["opp_c"].any())
    from concourse.bass_utils import run_bass_kernel_spmd
    if zero_state:
        packed = pack_fast(inputs, np_dt)
        nc = build_fast(BC, np_dt)
        in_maps = []
        for c in range(NCORE):
            m = {}
            for k, v in packed.items():
                if k == "xT32":
                    m[k] = np.ascontiguousarray(v[:, c * BC:(c + 1) * BC])
                else:
                    m[k] = v
            in_maps.append(m)
        res = run_bass_kernel_spmd(nc, in_maps, list(range(NCORE)))
        outs = [res.results[c]["out16"].reshape(-1) for c in range(NCORE)]
        return np.concatenate(outs).reshape(B, 1).astype(np.float32)

    # general fallback (nonzero states): original slot kernel
    FD = 2048
    packed = pack_host(inputs, np_dt)
    nc = build_nc(BC, FD, np_dt)
    batch_keys = [k for k in packed if k.startswith(("st", "ct"))]
    in_maps = []
    for c in range(NCORE):
        m = {}
        for k, v in packed.items():
            if k in batch_keys:
                m[k] = np.ascontiguousarray(v[:, c * BC:(c + 1) * BC])
            else:
                m[k] = v
        in_maps.append(m)
    res = run_bass_kernel_spmd(nc, in_maps, list(range(NCORE)))
    outs = [res.results[c]["out"].reshape(-1) for c in range(NCORE)]
    return np.concatenate(outs).reshape(B, 1).astype(np.float32)


if __name__ == "__main__":
    pass
